# revision 1
# baseline (speedup 1.0000x reference)
"""BiAttention TRN2 kernel: data-parallel over batch across 8 NeuronCores.

Self-contained: hardcodes B=32, Tc=2048, Tq=256, D=256, 8 cores, 4 batches/core.
Raw-bass software-pipelined kernel; f32r matmuls; exact power-of-two mask trick.
"""
import numpy as np

import concourse.bass as bass
from concourse import mybir
from concourse.bass_utils import run_bass_kernel_spmd

F32 = mybir.dt.float32
F32R = mybir.dt.float32r
BF16 = mybir.dt.bfloat16
Exp = mybir.ActivationFunctionType.Exp
AX = mybir.AxisListType
OP = mybir.AluOpType

B, TC, TQ, D = 32, 2048, 256, 256
NCORES = 8
NB = B // NCORES          # batches per core = 4
NBLK = TC // 128          # c-blocks per batch = 16
NEG = -(2.0 ** 96)
SQ = 2.0 ** 48


def build_program():
    nc = bass.Bass()
    c_d = nc.declare_dram_parameter("c", [NB, TC, D], F32, isOutput=False)
    q_d = nc.declare_dram_parameter("q", [NB, TQ, D], F32, isOutput=False)
    mc_d = nc.declare_dram_parameter("mcf", [NB, 2, TC], F32, isOutput=False)
    mq_d = nc.declare_dram_parameter("mqf", [NB, 2, TQ], F32, isOutput=False)
    id_d = nc.declare_dram_parameter("ident", [128, 128], F32, isOutput=False)
    onew_d = nc.declare_dram_parameter("onesw", [128, 256], F32, isOutput=False)
    c100_d = nc.declare_dram_parameter("c100", [128, 1], F32, isOutput=False)

    o_d = nc.declare_dram_parameter("o", [NB, TC, D], F32, isOutput=True)
    qc_d = nc.declare_dram_parameter("qc", [NB, TQ], F32, isOutput=True)

    from contextlib import ExitStack
    es = ExitStack()
    _ctr = [0]

    def sb(shape, dt, name=None):
        _ctr[0] += 1
        return es.enter_context(nc.sbuf_tensor(name or f"sb{_ctr[0]}", shape, dt))

    def ps(shape, dt, name=None):
        _ctr[0] += 1
        return es.enter_context(nc.psum_tensor(name or f"ps{_ctr[0]}", shape, dt))

    def sem(name):
        return es.enter_context(nc.semaphore(name))

    # ---- SBUF ----
    cb = [sb([128, NBLK, D], F32R) for _ in range(2)]      # C natural (f32r), per-batch parity
    qn = [sb([128, 2, D], F32R) for _ in range(2)]          # Q natural [q%128, qchunk, d]
    qtr = [sb([128, 2, TQ], F32R) for _ in range(2)]        # Q^T [d%128, dchunk, q]
    mcs = [sb([2, TC], F32R) for _ in range(2)]             # mask lhsT features
    mqs = [sb([2, TQ], F32R) for _ in range(2)]             # mask rhs features
    ident = sb([128, 128], F32R)
    ones_w = sb([128, 256], F32R)                           # all-ones (total-sum rhs)
    c100 = sb([128, 1], F32)                                # bias constant -100
    ctr = [sb([128, 2, 2, 128], F32R) for _ in range(2)]    # C^T (par, chunk, c), pair-parity
    ptr = [sb([128, 2, 2, 128], BF16) for _ in range(2)]    # P^T (par, chunk, c), pair-parity
    p_sb = [sb([128, TQ], BF16) for _ in range(4)]          # exp(S-m) (bf16), 4-deep
    qn_b = [sb([128, 2, D], BF16) for _ in range(2)]        # Q natural bf16 (mm2 rhs)
    ident_b = sb([128, 128], BF16)
    o_all = [sb([128, NBLK, D], F32) for _ in range(2)]     # output batch buffer
    NM = [sb([128, NBLK], F32) for _ in range(2)]           # -rowmax per block column
    SS = [sb([128, NBLK], F32) for _ in range(2)]           # rowsum per block column
    RS = [sb([128, NBLK], F32) for _ in range(2)]           # 1/rowsum
    E_all = sb([128, NBLK], F32R)                           # exp(m - 100) for q2c
    esum = sb([128, 1], F32)
    esum_r = sb([128, 1], F32R)
    t_sb = sb([1, 1], F32)
    rtot = sb([1, 1], F32)
    qc_sb = [sb([1, TQ], F32) for _ in range(2)]

    # ---- PSUM (bank-granular allocator: 8 banks total) ----
    pJ = [ps([128, 2, 256], F32R) for _ in range(2)]  # C^T pair buffers (1 bank each)
    pPT = ps([128, 256], F32R)                      # P^T both parities (bf16 via bitcast), 1 bank
    pS = ps([128, 4, 256], F32)                     # sim quad (2 banks); QT prep borrows bank0 via f32r bitcast
    pO = [ps([128, 256], F32) for _ in range(2)]    # mm2 out, 1 bank each
    # pM regions: pQC=[0:1,0:256], pTot=[0:1,256:512]
    pM = ps([128, 512], F32)

    sems = {}
    for name in ("s_cin", "s_out", "s_qc", "pe_ct", "pe_qt", "pe_s", "pe_pt",
                 "pe_o", "pt_", "dve_ctr", "dve_qtr", "dve_nm", "dve_rs",
                 "dve_ptr", "dt", "act_p", "act_o", "at", "s_misc"):
        sems[name] = sem(name)
    s_cin = sems["s_cin"]; s_out = sems["s_out"]; s_qc = sems["s_qc"]
    pe_ct = sems["pe_ct"]; pe_qt = sems["pe_qt"]; pe_s = sems["pe_s"]
    pe_pt = sems["pe_pt"]; pe_o = sems["pe_o"]; pt_ = sems["pt_"]
    dve_ctr = sems["dve_ctr"]; dve_qtr = sems["dve_qtr"]; dve_nm = sems["dve_nm"]
    dve_rs = sems["dve_rs"]; dve_ptr = sems["dve_ptr"]; dt = sems["dt"]
    act_p = sems["act_p"]; act_o = sems["act_o"]; at = sems["at"]
    s_misc = sems["s_misc"]

    blk = es.enter_context(nc.Block())
    with blk:
        # ---------------- GPSIMD: input cast-DMAs ----------------
        @blk.gpsimd
        def _(g):
            for b in range(NB):
                if b >= 2:
                    g.wait_ge(pt_, b - 1)
                if b >= 1:
                    # all previously issued input DMAs must have completed so
                    # cumulative thresholds are meaningful (unordered DMA completion)
                    g.wait_ge(s_cin, 64 * b + 48)
                g.dma_start(cb[b % 2][:], c_d[b].rearrange("(i p) d -> p i d", p=128)).then_inc(s_cin, 16)
                g.dma_start(qn[b % 2][:], q_d[b].rearrange("(a p) d -> p a d", p=128)).then_inc(s_cin, 16)
                g.dma_start(mcs[b % 2][:], mc_d[b]).then_inc(s_cin, 16)
                g.dma_start(mqs[b % 2][:], mq_d[b]).then_inc(s_cin, 16)
                if b == 0:
                    g.dma_start(ident[:], id_d[:]).then_inc(s_cin, 16)
                    g.dma_start(ones_w[:], onew_d[:]).then_inc(s_cin, 16)
                    g.dma_start(c100[:], c100_d[:]).then_inc(s_cin, 16)

        def cin_thresh(b):
            return 64 * (b + 1) + 48

        # ---------------- PE ----------------
        @blk.tensor
        def _(t):
            def ct_tr(n):
                b, i = divmod(n, NBLK)
                k = n % 2
                if i == 0:
                    t.wait_ge(s_cin, cin_thresh(b))
                pp = (n // 2) % 2
                tr0 = t.transpose(pJ[pp][:, k, 0:128], cb[b % 2][:, i, 0:128], ident[:])
                if n >= 4:
                    tr0._wait_ge(dve_ctr, n // 2 - 1)   # pair copy 2 pairs back done
                t.transpose(pJ[pp][:, k, 128:256], cb[b % 2][:, i, 128:256], ident[:]).then_inc(pe_ct, 1)

            def sim(n):
                b, i = divmod(n, NBLK)
                k = n % 2
                q = n % 4
                t.wait_ge(dve_ctr, n // 2 + 1)
                if i in (0, 1):
                    t.wait_ge(dve_qtr, b + 1)     # bank0 quarters held QT
                ap = n - 2 - (n % 2)              # exp of evicted/conflicting quarter done
                if ap >= 1:
                    t.wait_ge(act_p, ap)          # also implies dve_nm >= n//4 transitively
                elif n >= 4:
                    t.wait_ge(dve_nm, n // 4)
                mm0 = t.matmul(pS[:, q, :], mcs[b % 2][:, i * 128:(i + 1) * 128],
                               mqs[b % 2][:], start=True, stop=False)
                pp = (n // 2) % 2
                t.matmul(pS[:, q, :], ctr[pp][:, k, 0], qtr[b % 2][:, 0], start=False, stop=False)
                t.matmul(pS[:, q, :], ctr[pp][:, k, 1], qtr[b % 2][:, 1], start=False, stop=True).then_inc(pe_s, 1)

            def pt_tr(n):
                k = n % 2
                if n == 0:
                    t.wait_ge(s_misc, 1)    # ident_b ready
                if n >= 2:
                    t.wait_ge(dve_ptr, n // 2)   # pair copy of (n-2) done (whole bank)
                ptb = pPT[:].bitcast(BF16)
                tr0 = t.transpose(ptb[:, k * 256:k * 256 + 128], p_sb[n % 4][:, 0:128], ident_b[:])
                tr0._wait_ge(act_p, n + 1)
                t.transpose(ptb[:, k * 256 + 128:k * 256 + 256], p_sb[n % 4][:, 128:256],
                            ident_b[:]).then_inc(pe_pt, 1)

            def mm2(n):
                b, i = divmod(n, NBLK)
                k = n % 2
                if n >= 2:
                    t.wait_ge(act_o, n - 1)   # outcp(n-2) done (own bank)
                pp = (n // 2) % 2
                mm0 = t.matmul(pO[k][:], ptr[pp][:, k, 0], qn_b[b % 2][:, 0], start=True, stop=False)
                mm0._wait_ge(dve_ptr, n // 2 + 1)
                t.matmul(pO[k][:], ptr[pp][:, k, 1], qn_b[b % 2][:, 1], start=False, stop=True).then_inc(pe_o, 1)

            def qt_prep(b):
                t.wait_ge(s_cin, cin_thresh(b))
                if b >= 1:
                    t.wait_ge(dve_qtr, b)       # prev QT copy done
                    t.wait_ge(act_p, 16 * b)    # pS bank0 prior exps done
                    t.wait_ge(dve_nm, 4 * b)    # prior quad reads done
                psr = pS[:].bitcast(F32R)
                last = None
                for qa in range(2):
                    for kk in range(2):
                        last = t.transpose(
                            psr[:, kk, qa * 128:(qa + 1) * 128],
                            qn[b % 2][:, qa, kk * 128:(kk + 1) * 128],
                            ident[:],
                        )
                last.then_inc(pe_qt, 1)

            def tail(b):
                # C: q2c matmuls + total sum (constant-shift exp, no global max)
                t.wait_ge(dt, 2 * b + 1)      # esum_r ready
                t.wait_ge(at, 2 * b + 1)      # E_all ready
                if b >= 1:
                    t.wait_ge(at, 2 * b)      # T2(b-1) done reading pM
                for i in range(NBLK):
                    t.matmul(pM[0:1, 0:256], E_all[:, i:i + 1], cb[b % 2][:, i, :],
                             start=(i == 0), stop=(i == NBLK - 1))
                t.matmul(pM[0:1, 256:512], esum_r[:], ones_w[:], start=True,
                         stop=True).then_inc(pt_, 1)

            for b in range(NB):
                qt_prep(b)
                for slot in range(NBLK + 12):
                    i = slot - 6
                    if 0 <= i <= NBLK - 1:
                        pt_tr(16 * b + i)
                    i = slot - 8
                    if 0 <= i <= NBLK - 1:
                        mm2(16 * b + i)
                    i = slot
                    if 0 <= i <= NBLK - 1:
                        ct_tr(16 * b + i)
                    i = slot - 2
                    if 0 <= i <= NBLK - 1:
                        sim(16 * b + i)
                tail(b)

        # ---------------- DVE ----------------
        @blk.vector
        def _(v):
            def qtr_copy(b):
                if b == 0:
                    v.wait_ge(s_cin, cin_thresh(0))
                    v.tensor_copy(ident_b[:], ident[:]).then_inc(s_misc, 1)
                v.wait_ge(pe_qt, b + 1)
                if b >= 2:
                    v.wait_ge(pe_o, 16 * (b - 1))   # qn_b WAR (implies pe_s too)
                v.tensor_copy(qn_b[b % 2][:], qn[b % 2][:])
                v.tensor_copy(qtr[b % 2][:], pS[:].bitcast(F32R)[:, 0:2, :]).then_inc(dve_qtr, 1)

            def ctr_pair(b, p):
                # copy C^T for blocks 16b+2p, +2p+1 in one op
                n1 = 16 * b + 2 * p + 1
                if n1 >= 5:
                    v.wait_ge(pe_s, n1 - 3)       # sims of pair evicted 2 pairs ago done
                cp = v.tensor_copy(ctr[p % 2][:], pJ[p % 2][:])
                cp._wait_ge(pe_ct, n1 + 1)
                cp.then_inc(dve_ctr, 1)

            def nm_quad(b, qq):
                # one reduce for blocks 16b+4qq .. +3
                i4 = 4 * qq
                if qq == 0 and b >= 2:
                    v.wait_ge(at, 2 * (b - 2) + 1)   # tail(b-2) E-exp read NM buffer
                rd = v.tensor_reduce(NM[b % 2][:, i4:i4 + 4], pS[:], AX.X, OP.max,
                                     negate=True)
                rd._wait_ge(pe_s, 16 * b + 4 * qq + 4)
                rd.then_inc(dve_nm, 1)

            def ptr_pair(b, p):
                n1 = 16 * b + 2 * p + 1
                if n1 >= 5:
                    v.wait_ge(pe_o, n1 - 3)       # mm2s of pair evicted 2 pairs ago done
                cp = v.tensor_copy(ptr[p % 2][:], pPT[:].bitcast(BF16)[:, 0:512])
                cp._wait_ge(pe_pt, n1 + 1)
                cp.then_inc(dve_ptr, 1)

            def recip(n):
                b, i = divmod(n, NBLK)
                if i == 0 and b >= 2:
                    v.wait_ge(act_o, 16 * (b - 1))   # RS WAR vs out-copy of b-2
                rc = v.reciprocal(RS[b % 2][:, i:i + 1], SS[b % 2][:, i:i + 1])
                rc._wait_ge(act_p, n + 1)
                rc.then_inc(dve_rs, 1)

            def tail(b):
                # X1: esum -> f32r
                v.wait_ge(at, 2 * b + 1)
                v.tensor_copy(esum_r[:], esum[:]).then_inc(dt, 1)
                # X2: total -> reciprocal
                v.wait_ge(pt_, b + 1)
                if b >= 1:
                    v.wait_ge(at, 2 * b)   # T2(b-1) done with rtot
                v.tensor_copy(t_sb[:], pM[0:1, 256:257])
                v.drain()
                v.reciprocal(rtot[:], t_sb[:]).then_inc(dt, 1)

            for b in range(NB):
                qtr_copy(b)
                for slot in range(NBLK + 12):
                    if slot >= 7 and slot % 2 == 1 and (slot - 7) // 2 <= 7:
                        ptr_pair(b, (slot - 7) // 2)
                    i = slot - 6
                    if 0 <= i <= NBLK - 1:
                        recip(16 * b + i)
                    if slot % 2 == 1 and (slot - 1) // 2 <= 7:
                        ctr_pair(b, (slot - 1) // 2)
                    if slot >= 5 and (slot - 5) % 4 == 0 and (slot - 5) // 4 <= 3:
                        nm_quad(b, (slot - 5) // 4)
                tail(b)

        # ---------------- ACT ----------------
        @blk.scalar
        def _(s):
            def ex(n):
                b, i = divmod(n, NBLK)
                q = n % 4
                if n >= 4:
                    s.wait_ge(pe_pt, n - 3)          # p_sb 4-deep WAR
                if i == 0 and b >= 2:
                    s.wait_ge(dve_rs, 16 * (b - 1))  # SS WAR vs recip of b-2
                ac = s.activation(p_sb[q][:], pS[:, q, :], Exp,
                                  bias=NM[b % 2][:, i:i + 1],
                                  accum_out=SS[b % 2][:, i:i + 1])
                ac._wait_ge(dve_nm, 4 * b + n % 16 // 4 + 1)
                ac.then_inc(act_p, 1)

            def outcp(n):
                b, i = divmod(n, NBLK)
                k = n % 2
                s.wait_ge(dve_rs, n + 1)
                if i == 0 and b >= 2:
                    s.wait_ge(s_out, 16 * (b - 1))
                oc = s.mul(o_all[b % 2][:, i, :], pO[k][:], RS[b % 2][:, i:i + 1])
                oc._wait_ge(pe_o, n + 1)
                oc.then_inc(act_o, 1)

            def tail(b):
                # T1: E = exp(-NM - 100), accum esum
                s.wait_ge(dve_nm, 4 * (b + 1))
                if b >= 1:
                    s.wait_ge(pt_, b)        # E_all/esum WAR vs tail C of b-1
                s.activation(E_all[:], NM[b % 2][:], Exp, bias=c100[:], scale=-1.0,
                             accum_out=esum[:]).then_inc(at, 1)
                # T2: qc = pQC * rtot
                s.wait_ge(dt, 2 * b + 2)
                s.wait_ge(pt_, b + 1)
                if b >= 2:
                    s.wait_ge(s_qc, 16 * (b - 1))
                s.mul(qc_sb[b % 2][:], pM[0:1, 0:256], rtot[:]).then_inc(at, 1)

            for b in range(NB):
                for slot in range(NBLK + 12):
                    i = slot - 9
                    if 0 <= i <= NBLK - 1:
                        outcp(16 * b + i)
                    i = slot - 4
                    if 0 <= i <= NBLK - 1:
                        ex(16 * b + i)
                tail(b)

        # ---------------- SYNC: output DMAs ----------------
        @blk.sync
        def _(sy):
            for b in range(NB):
                sy.wait_ge(act_o, 16 * (b + 1))
                if b >= 1:
                    sy.wait_ge(s_out, 16 * b)
                sy.dma_start(o_d[b].rearrange("(i p) d -> p i d", p=128),
                             o_all[b % 2][:]).then_inc(s_out, 16)
                sy.wait_ge(at, 2 * b + 2)
                if b >= 1:
                    sy.wait_ge(s_qc, 16 * b)
                sy.dma_start(qc_d[b:b + 1, :], qc_sb[b % 2][:]).then_inc(s_qc, 16)

    return nc, es


_CACHE = {}


def _get_program():
    if "nc" not in _CACHE:
        nc, es = build_program()
        _CACHE["nc"] = nc
        _CACHE["es"] = es
    return _CACHE["nc"]


def kernel(context_repr, question_repr, context_len, question_len):
    context_repr = np.ascontiguousarray(np.asarray(context_repr, np.float32))
    question_repr = np.ascontiguousarray(np.asarray(question_repr, np.float32))
    context_len = np.asarray(context_len, np.int32)
    question_len = np.asarray(question_len, np.int32)

    cm = (np.arange(TC)[None, :] < context_len[:, None]).astype(np.float32)  # [B,Tc]
    qm = (np.arange(TQ)[None, :] < question_len[:, None]).astype(np.float32)  # [B,Tq]
    mcf = np.stack([SQ * cm, np.ones_like(cm)], axis=1)                      # [B,2,Tc]
    mqf = np.stack([SQ * qm, np.full_like(qm, NEG)], axis=1)                 # [B,2,Tq]
    ident = np.eye(128, dtype=np.float32)
    onesw = np.ones((128, 256), np.float32)
    c100 = np.full((128, 1), -100.0, np.float32)

    nc = _get_program()
    in_maps = []
    for core in range(NCORES):
        sl = slice(core * NB, (core + 1) * NB)
        in_maps.append({
            "c": context_repr[sl],
            "q": question_repr[sl],
            "mcf": np.ascontiguousarray(mcf[sl]),
            "mqf": np.ascontiguousarray(mqf[sl]),
            "ident": ident,
            "onesw": onesw,
            "c100": c100,
        })

    res = run_bass_kernel_spmd(nc, in_maps, list(range(NCORES)))
    out1 = np.concatenate([np.asarray(r["o"]).reshape(NB, TC, D) for r in res.results], axis=0)
    q2c = np.concatenate([np.asarray(r["qc"]).reshape(NB, TQ) for r in res.results], axis=0)
    out2 = np.ascontiguousarray(np.broadcast_to(q2c[:, None, :], (B, TC, D)))
    return out1, out2



# revision 5
# speedup vs baseline: 1.0623x; 1.0623x over previous
"""BiAttention TRN2 kernel: data-parallel over batch across 8 NeuronCores.

Self-contained: hardcodes B=32, Tc=2048, Tq=256, D=256, 8 cores, 4 batches/core.

v2 design (vs baseline):
- Host pre-transposes C and Q: device receives C^T f32 (sim lhsT), C bf16
  (q2c lhsT), Q^T f32 (sim rhs), Q bf16 with a ones column (mm2 rhs).
  This removes all C/Q transposes + PSUM->SBUF staging from PE/DVE.
- Row sums come free from the ones column of the mm2 rhs (out col 256),
  so exp needs no accumulator read.
- q2c computed as 1-row-moving matmuls (out [128,1]) -- near-zero PE cost;
  normalization by the total happens on host.
- Output is stored bf16 (halves O DMA traffic); normalize-muls split
  across ACT (even blocks) and DVE (odd blocks); row-max reductions on
  GPSIMD (Pool); all DMAs issued from SP (sync) HWDGE queues.
"""
import numpy as np
import ml_dtypes

import concourse.bass as bass
from concourse import mybir
from concourse.bass_utils import run_bass_kernel_spmd

F32 = mybir.dt.float32
F32R = mybir.dt.float32r
BF16 = mybir.dt.bfloat16
Exp = mybir.ActivationFunctionType.Exp
AX = mybir.AxisListType
OP = mybir.AluOpType

B, TC, TQ, D = 32, 2048, 256, 256
NCORES = 8
NB = B // NCORES          # batches per core = 4
NBLK = TC // 128          # c-blocks per batch = 16
NEG = -(2.0 ** 96)
SQ = 2.0 ** 48
QN_W = TQ + 1             # mm2 rhs width: D cols of Q + ones column


def build_program():
    nc = bass.Bass()
    ct_d = nc.declare_dram_parameter("ct", [NB, 2, 128, TC], F32R, isOutput=False)
    cn_d = nc.declare_dram_parameter("cn", [NB, TC, D], BF16, isOutput=False)
    qt_d = nc.declare_dram_parameter("qt", [NB, 2, 128, TQ], F32R, isOutput=False)
    qn_d = nc.declare_dram_parameter("qn", [NB, 2, 128, QN_W], BF16, isOutput=False)
    mc_d = nc.declare_dram_parameter("mcf", [NB, 2, TC], F32R, isOutput=False)
    mq_d = nc.declare_dram_parameter("mqf", [NB, 2, TQ], F32R, isOutput=False)
    id_d = nc.declare_dram_parameter("identb", [128, 128], BF16, isOutput=False)
    c100_d = nc.declare_dram_parameter("c100", [128, 1], F32, isOutput=False)
    ones_d = nc.declare_dram_parameter("ones128", [128, 1], F32, isOutput=False)

    o_d = nc.declare_dram_parameter("o", [NB, TC, D], BF16, isOutput=True)
    qc_d = nc.declare_dram_parameter("qc", [NB, 128, 3], F32, isOutput=True)

    from contextlib import ExitStack
    es = ExitStack()
    _ctr = [0]

    def sb(shape, dt, name=None):
        _ctr[0] += 1
        return es.enter_context(nc.sbuf_tensor(name or f"sb{_ctr[0]}", shape, dt))

    def ps(shape, dt, name=None):
        _ctr[0] += 1
        return es.enter_context(nc.psum_tensor(name or f"ps{_ctr[0]}", shape, dt))

    def sem(name):
        return es.enter_context(nc.semaphore(name))

    # ---- SBUF ----
    ctr = [sb([128, 2, TC], F32R) for _ in range(2)]    # C^T [d%128, dchunk, c]
    cbn = [sb([128, NBLK, D], BF16) for _ in range(2)]  # C natural bf16
    qtr = [sb([128, 2, TQ], F32R) for _ in range(2)]    # Q^T [d%128, dchunk, q]
    qnb = [sb([128, 2, QN_W], BF16) for _ in range(2)]  # Q nat + ones col [q%128, qhalf, d]
    mcs = [sb([2, TC], F32R) for _ in range(2)]         # mask lhsT features
    mqs = [sb([2, TQ], F32R) for _ in range(2)]         # mask rhs features
    identb = sb([128, 128], BF16)
    c100 = sb([128, 1], F32)                            # bias constant -100
    ones128 = sb([128, 1], F32)
    p_sb = [sb([128, TQ], BF16) for _ in range(4)]      # exp(S-m) (bf16), 4-deep
    ptr = [sb([128, 2, 2, 128], BF16) for _ in range(2)]  # P^T (q, blkpar, qhalf, c)
    NM = [sb([128, NBLK], F32) for _ in range(2)]       # -rowmax per block column
    RS = [sb([128, NBLK], F32) for _ in range(2)]       # 1/rowsum
    E_all = [sb([128, NBLK], BF16) for _ in range(2)]   # exp(m - 100) for q2c
    esum = [sb([128, 1], F32) for _ in range(2)]
    o_sb = [sb([128, NBLK, D], BF16) for _ in range(2)]  # output batch buffer
    qc_sb = [sb([128, 3], F32) for _ in range(2)]       # staged q2cT + total

    # ---- PSUM (8 banks) ----
    pS = ps([128, 8, 256], F32)       # sim, 2 quads (4 banks); slot = n%8
    pPT = ps([128, 256], F32R)        # P^T pair (bf16 via bitcast), 1 bank
    pO = [ps([128, QN_W], F32) for _ in range(2)]  # mm2 out (+rowsum col), 1 bank each
    pM = ps([128, 512], F32)          # q2cT cols 0:2, total at [0:1, 8:9]

    sems = {}
    for name in ("s_cin", "s_out", "s_qc", "pe_s", "pe_pt", "pe_o", "pt_",
                 "dve_nm", "act_p", "act_o", "dve_ptr", "dve_rs",
                 "at", "dv_qc"):
        sems[name] = sem(name)
    s_cin = sems["s_cin"]; s_out = sems["s_out"]; s_qc = sems["s_qc"]
    pe_s = sems["pe_s"]; pe_pt = sems["pe_pt"]; pe_o = sems["pe_o"]
    pt_ = sems["pt_"]; dve_nm = sems["dve_nm"]; act_p = sems["act_p"]
    act_o = sems["act_o"]
    dve_ptr = sems["dve_ptr"]; dve_rs = sems["dve_rs"]; at = sems["at"]
    dv_qc = sems["dv_qc"]

    def cin_thresh(b):
        return 96 * (b + 1) + 48

    blk = es.enter_context(nc.Block())
    with blk:
        # ---------------- SP: all DMAs ----------------
        @blk.sync
        def _(sy):
            def issue_inputs(b):
                if b >= 2:
                    # drain: all previously issued input DMAs complete so
                    # cumulative thresholds are meaningful
                    sy.wait_ge(s_cin, 96 * b + 48)
                    # WAR: batch b-2 consumers done with the b%2 buffers
                    sy.wait_ge(pe_s, 16 * (b - 1))
                    sy.wait_ge(pe_o, 16 * (b - 1))
                    sy.wait_ge(pt_, b - 1)
                sy.dma_start(ctr[b % 2][:],
                             ct_d[b].rearrange("k p c -> p k c")).then_inc(s_cin, 16)
                sy.dma_start(cbn[b % 2][:],
                             cn_d[b].rearrange("(i p) d -> p i d", p=128)).then_inc(s_cin, 16)
                sy.dma_start(qtr[b % 2][:],
                             qt_d[b].rearrange("k p q -> p k q")).then_inc(s_cin, 16)
                sy.dma_start(qnb[b % 2][:],
                             qn_d[b].rearrange("k p d -> p k d")).then_inc(s_cin, 16)
                sy.dma_start(mcs[b % 2][:], mc_d[b]).then_inc(s_cin, 16)
                sy.dma_start(mqs[b % 2][:], mq_d[b]).then_inc(s_cin, 16)

            sy.dma_start(identb[:], id_d[:]).then_inc(s_cin, 16)
            sy.dma_start(c100[:], c100_d[:]).then_inc(s_cin, 16)
            sy.dma_start(ones128[:], ones_d[:]).then_inc(s_cin, 16)
            issue_inputs(0)
            issue_inputs(1)
            for b in range(NB):
                if b + 2 < NB:
                    issue_inputs(b + 2)
                sy.wait_ge(act_o, 16 * (b + 1))
                if b >= 2:
                    sy.wait_ge(s_out, 16 * (b - 1))
                sy.dma_start(o_d[b].rearrange("(i p) d -> p i d", p=128),
                             o_sb[b % 2][:]).then_inc(s_out, 16)
                sy.wait_ge(dv_qc, b + 1)
                sy.dma_start(qc_d[b], qc_sb[b % 2][:]).then_inc(s_qc, 16)

        # ---------------- PE ----------------
        @blk.tensor
        def _(t):
            def sim(n):
                b, i = divmod(n, NBLK)
                sl = n % 8
                if i == 0:
                    t.wait_ge(s_cin, cin_thresh(b))
                if n >= 8:
                    t.wait_ge(act_p, n - 7)   # exp(n-8) done -> pS slot free
                t.matmul(pS[:, sl, :], mcs[b % 2][:, 128 * i:128 * (i + 1)],
                         mqs[b % 2][:], start=True, stop=False)
                t.matmul(pS[:, sl, :], ctr[b % 2][:, 0, 128 * i:128 * (i + 1)],
                         qtr[b % 2][:, 0, :], start=False, stop=False)
                t.matmul(pS[:, sl, :], ctr[b % 2][:, 1, 128 * i:128 * (i + 1)],
                         qtr[b % 2][:, 1, :], start=False, stop=True).then_inc(pe_s, 1)

            def pt_tr(n):
                k = n % 2
                if n >= 2:
                    t.wait_ge(dve_ptr, n // 2)   # pair copy 2 blocks back done
                ptb = pPT[:].bitcast(BF16)
                tr0 = t.transpose(ptb[:, k * 256:k * 256 + 128],
                                  p_sb[n % 4][:, 0:128], identb[:])
                tr0._wait_ge(act_p, n + 1)
                t.transpose(ptb[:, k * 256 + 128:k * 256 + 256],
                            p_sb[n % 4][:, 128:256], identb[:]).then_inc(pe_pt, 1)

            def mm2(n):
                b, i = divmod(n, NBLK)
                k = n % 2
                pp = (n // 2) % 2
                if n >= 2:
                    t.wait_ge(act_o, n - 1)            # outcp(n-2) done
                    t.wait_ge(dve_rs, n - 1)           # recip(n-2) done
                mm0 = t.matmul(pO[k][:], ptr[pp][:, k, 0], qnb[b % 2][:, 0, :],
                               start=True, stop=False)
                mm0._wait_ge(dve_ptr, n // 2 + 1)
                t.matmul(pO[k][:], ptr[pp][:, k, 1], qnb[b % 2][:, 1, :],
                         start=False, stop=True).then_inc(pe_o, 1)

            def tail(b):
                t.wait_ge(at, b + 1)          # E_all/esum ready
                if b >= 1:
                    t.wait_ge(dv_qc, b)       # qc staging of b-1 done (pM free)
                for dh in range(2):
                    for i in range(NBLK):
                        t.matmul(pM[:, dh:dh + 1],
                                 cbn[b % 2][:, i, 128 * dh:128 * (dh + 1)],
                                 E_all[b % 2][:, i:i + 1],
                                 start=(i == 0), stop=(i == NBLK - 1))
                t.matmul(pM[0:1, 8:9], esum[b % 2][:], ones128[:],
                         start=True, stop=True).then_inc(pt_, 1)

            for b in range(NB):
                for slot in range(NBLK + 11):
                    i = slot - 6
                    if 0 <= i < NBLK:
                        pt_tr(16 * b + i)
                    i = slot - 8
                    if 0 <= i < NBLK:
                        mm2(16 * b + i)
                    i = slot
                    if 0 <= i < NBLK:
                        sim(16 * b + i)
                tail(b)

        # ---------------- ACT ----------------
        @blk.scalar
        def _(s):
            def ex(n):
                b, i = divmod(n, NBLK)
                sl = n % 8
                if n >= 4:
                    s.wait_ge(pe_pt, n - 3)   # p_sb 4-deep WAR
                ac = s.activation(p_sb[n % 4][:], pS[:, sl, :], Exp,
                                  bias=NM[b % 2][:, i:i + 1])
                ac._wait_ge(dve_nm, 4 * b + i // 4 + 1)
                ac.then_inc(act_p, 1)

            def outcp(n):
                b, i = divmod(n, NBLK)
                k = n % 2
                s.wait_ge(dve_rs, n + 1)
                if i == 0 and b >= 2:
                    s.wait_ge(s_out, 16 * (b - 1))
                s.mul(o_sb[b % 2][:, i, :], pO[k][:, 0:256],
                      RS[b % 2][:, i:i + 1]).then_inc(act_o, 1)

            def t1(b):
                s.wait_ge(dve_nm, 4 * (b + 1))
                if b >= 2:
                    s.wait_ge(pt_, b - 1)     # tail(b-2) done reading E/esum
                s.activation(E_all[b % 2][:], NM[b % 2][:], Exp, bias=c100[:],
                             scale=-1.0, accum_out=esum[b % 2][:]).then_inc(at, 1)

            for b in range(NB):
                for slot in range(NBLK + 11):
                    i = slot - 11
                    if 0 <= i < NBLK:
                        outcp(16 * b + i)
                    i = slot - 5
                    if 0 <= i < NBLK:
                        ex(16 * b + i)
                t1(b)

        # ---------------- DVE ----------------
        @blk.vector
        def _(v):
            def ptr_pair(b, p):
                n1 = 16 * b + 2 * p + 1
                if n1 >= 5:
                    v.wait_ge(pe_o, n1 - 3)   # mm2s of pair evicted 2 pairs ago
                cp = v.tensor_copy(ptr[p % 2][:], pPT[:].bitcast(BF16)[:, 0:512])
                cp._wait_ge(pe_pt, n1 + 1)
                cp.then_inc(dve_ptr, 1)

            def nm_quad(b, qq):
                if qq == 0 and b >= 2:
                    v.wait_ge(at, b - 1)   # T1(b-2) done reading NM[b%2]
                sl4 = ((4 * b + qq) % 2) * 4
                rd = v.tensor_reduce(NM[b % 2][:, 4 * qq:4 * qq + 4],
                                     pS[:, sl4:sl4 + 4, :], AX.X, OP.max,
                                     negate=True)
                rd._wait_ge(pe_s, 16 * b + 4 * qq + 4)
                rd.then_inc(dve_nm, 1)

            def recip(n):
                b, i = divmod(n, NBLK)
                k = n % 2
                if i == 0 and b >= 2:
                    v.wait_ge(act_o, 16 * (b - 1))   # RS[b%2] WAR
                rc = v.reciprocal(RS[b % 2][:, i:i + 1], pO[k][:, 256:257])
                rc._wait_ge(pe_o, n + 1)
                rc.then_inc(dve_rs, 1)

            def tail(b):
                v.wait_ge(pt_, b + 1)
                if b >= 2:
                    v.wait_ge(s_qc, 16 * (b - 1))    # qc DMA(b-2) done
                v.tensor_copy(qc_sb[b % 2][:, 0:2], pM[:, 0:2])
                v.tensor_copy(qc_sb[b % 2][0:1, 2:3],
                              pM[0:1, 8:9]).then_inc(dv_qc, 1)

            for b in range(NB):
                for slot in range(NBLK + 11):
                    if slot >= 7 and slot % 2 == 1 and (slot - 7) // 2 <= 7:
                        ptr_pair(b, (slot - 7) // 2)
                    i = slot - 10
                    if 0 <= i < NBLK:
                        recip(16 * b + i)
                    if slot >= 5 and (slot - 5) % 4 == 0 and (slot - 5) // 4 <= 3:
                        nm_quad(b, (slot - 5) // 4)
                tail(b)

    return nc, es


_CACHE = {}


def _get_program():
    if "nc" not in _CACHE:
        nc, es = build_program()
        _CACHE["nc"] = nc
        _CACHE["es"] = es
    return _CACHE["nc"]


def kernel(context_repr, question_repr, context_len, question_len):
    C = np.ascontiguousarray(np.asarray(context_repr, np.float32))
    Q = np.ascontiguousarray(np.asarray(question_repr, np.float32))
    context_len = np.asarray(context_len, np.int32)
    question_len = np.asarray(question_len, np.int32)
    bf16 = ml_dtypes.bfloat16

    cm = (np.arange(TC)[None, :] < context_len[:, None]).astype(np.float32)
    qm = (np.arange(TQ)[None, :] < question_len[:, None]).astype(np.float32)
    mcf = np.ascontiguousarray(np.stack([SQ * cm, np.ones_like(cm)], axis=1))
    mqf = np.ascontiguousarray(np.stack([SQ * qm, np.full_like(qm, NEG)], axis=1))

    ct = np.ascontiguousarray(C.transpose(0, 2, 1)).reshape(B, 2, 128, TC)
    cn = C.astype(bf16)
    qt = np.ascontiguousarray(Q.transpose(0, 2, 1)).reshape(B, 2, 128, TQ)
    qn = np.concatenate([Q, np.ones((B, TQ, 1), np.float32)], axis=2)
    qn = np.ascontiguousarray(qn.reshape(B, 2, 128, QN_W).astype(bf16))
    identb = np.eye(128, dtype=bf16)
    c100 = np.full((128, 1), -100.0, np.float32)
    ones128 = np.ones((128, 1), np.float32)

    nc = _get_program()
    in_maps = []
    for core in range(NCORES):
        sl = slice(core * NB, (core + 1) * NB)
        in_maps.append({
            "ct": np.ascontiguousarray(ct[sl]),
            "cn": np.ascontiguousarray(cn[sl]),
            "qt": np.ascontiguousarray(qt[sl]),
            "qn": np.ascontiguousarray(qn[sl]),
            "mcf": np.ascontiguousarray(mcf[sl]),
            "mqf": np.ascontiguousarray(mqf[sl]),
            "identb": identb,
            "c100": c100,
            "ones128": ones128,
        })

    res = run_bass_kernel_spmd(nc, in_maps, list(range(NCORES)))
    out1 = np.concatenate(
        [np.asarray(r["o"]).reshape(NB, TC, D).astype(np.float32)
         for r in res.results], axis=0)
    qc_raw = np.concatenate(
        [np.asarray(r["qc"]).reshape(NB, 128, 3) for r in res.results], axis=0)
    q2c = qc_raw[:, :, 0:2].transpose(0, 2, 1).reshape(B, D) / qc_raw[:, 0:1, 2]
    out2 = np.ascontiguousarray(np.broadcast_to(q2c[:, None, :], (B, TC, D)))
    return out1, out2


# revision 7
# speedup vs baseline: 1.4960x; 1.4083x over previous
"""BiAttention TRN2 kernel: data-parallel over batch across 8 NeuronCores.

Self-contained: hardcodes B=32, Tc=2048, Tq=256, D=256, 8 cores, 4 batches/core.

Design:
- Host pre-transposes C and Q: device receives C^T f32 (sim lhsT), C bf16
  (q2c lhsT), Q^T f32 (sim rhs), Q bf16 with a ones column (mm2 rhs).
  No C/Q transposes or PSUM->SBUF staging on PE/DVE.
- Row sums come free from the ones column of the mm2 rhs (out col 256),
  so exp needs no accumulator read.
- q2c computed as 1-row-moving matmuls (out [128,1]) -- near-zero PE cost;
  normalization by the total happens on host.
- Output stored bf16 (halves O DMA); normalize-muls split 3:1 ACT/DVE;
  row-max quad reductions on DVE; all DMAs issued from SP HWDGE queues.
- All engine threads run one continuous software-pipelined stream over the
  64 global blocks (no per-batch barriers); deep rings (pS 6 slots, p_sb 6,
  dual P^T PSUM banks, pO ring 3) keep cross-engine feedback loops slack.
"""
import numpy as np
import ml_dtypes

import concourse.bass as bass
from concourse import mybir
from concourse.bass_utils import run_bass_kernel_spmd

F32 = mybir.dt.float32
F32R = mybir.dt.float32r
BF16 = mybir.dt.bfloat16
Exp = mybir.ActivationFunctionType.Exp
AX = mybir.AxisListType
OP = mybir.AluOpType

B, TC, TQ, D = 32, 2048, 256, 256
NCORES = 8
NB = B // NCORES          # batches per core = 4
NBLK = TC // 128          # c-blocks per batch = 16
NTOT = NB * NBLK          # total blocks = 64
NEG = -(2.0 ** 96)
SQ = 2.0 ** 48
QN_W = TQ + 1             # mm2 rhs width: D cols of Q + ones column

# pipeline stage lags (in global slots)
L_EX = 5
L_PT = 7
L_MM = 10
L_RC = 12
L_OC = 13
NSLOT = NTOT + L_OC + 2


def outcp_on_dve(n):
    return n % 4 == 3


def cnt_a(m):
    """# of outcp indices 0..m handled by ACT (n%4 != 3)."""
    return (m // 4) * 3 + min(m % 4 + 1, 3)


def cnt_d(m):
    """# of outcp indices 0..m handled by DVE (n%4 == 3)."""
    return m // 4 + (1 if m % 4 == 3 else 0)


def build_program():
    nc = bass.Bass()
    ct_d = nc.declare_dram_parameter("ct", [NB, 2, 128, TC], F32R, isOutput=False)
    cn_d = nc.declare_dram_parameter("cn", [NB, TC, D], BF16, isOutput=False)
    qt_d = nc.declare_dram_parameter("qt", [NB, 2, 128, TQ], F32R, isOutput=False)
    qn_d = nc.declare_dram_parameter("qn", [NB, 2, 128, QN_W], BF16, isOutput=False)
    mc_d = nc.declare_dram_parameter("mcf", [NB, 2, TC], F32R, isOutput=False)
    mq_d = nc.declare_dram_parameter("mqf", [NB, 2, TQ], F32R, isOutput=False)
    id_d = nc.declare_dram_parameter("identb", [128, 128], BF16, isOutput=False)
    c100_d = nc.declare_dram_parameter("c100", [128, 1], F32, isOutput=False)
    ones_d = nc.declare_dram_parameter("ones128", [128, 1], F32, isOutput=False)

    o_d = nc.declare_dram_parameter("o", [NB, TC, D], BF16, isOutput=True)
    qc_d = nc.declare_dram_parameter("qc", [NB, 128, 3], F32, isOutput=True)

    from contextlib import ExitStack
    es = ExitStack()
    _ctr = [0]

    def sb(shape, dt, name=None):
        _ctr[0] += 1
        return es.enter_context(nc.sbuf_tensor(name or f"sb{_ctr[0]}", shape, dt))

    def ps(shape, dt, name=None):
        _ctr[0] += 1
        return es.enter_context(nc.psum_tensor(name or f"ps{_ctr[0]}", shape, dt))

    def sem(name):
        return es.enter_context(nc.semaphore(name))

    # ---- SBUF ----
    ctr = [sb([128, 2, TC], F32R) for _ in range(2)]    # C^T [d%128, dchunk, c]
    cbn = [sb([128, NBLK, D], BF16) for _ in range(2)]  # C natural bf16
    qtr = [sb([128, 2, TQ], F32R) for _ in range(2)]    # Q^T [d%128, dchunk, q]
    qnb = [sb([128, 2, QN_W], BF16) for _ in range(2)]  # Q nat + ones col
    mcs = [sb([2, TC], F32R) for _ in range(2)]         # mask lhsT features
    mqs = [sb([2, TQ], F32R) for _ in range(2)]         # mask rhs features
    identb = sb([128, 128], BF16)
    c100 = sb([128, 1], F32)                            # bias constant -100
    ones128 = sb([128, 1], F32)
    p_sb = [sb([128, TQ], BF16) for _ in range(6)]      # exp(S-m) (bf16), 6-deep
    ptr = [sb([128, 2, 2, 128], BF16) for _ in range(2)]  # P^T (q, blkpar, qhalf, c)
    NM = [sb([128, NBLK], F32) for _ in range(2)]       # -rowmax per block column
    RS = [sb([128, NBLK], F32) for _ in range(2)]       # 1/rowsum
    E_all = [sb([128, NBLK], BF16) for _ in range(2)]   # exp(m - 100) for q2c
    esum = [sb([128, 1], F32) for _ in range(2)]
    o_sb = [sb([128, NBLK, D], BF16) for _ in range(2)]  # output batch buffer
    qc_sb = [sb([128, 3], F32) for _ in range(2)]       # staged q2cT + total

    # ---- PSUM (8 banks) ----
    pS = ps([128, 6, 256], F32)       # sim ring, 6 slots (3 banks)
    # P^T pair banks: lower half (f32 cols 0:256) holds bf16 P^T pairs via
    # bitcast; upper half of bank 1 doubles as the q2c accumulator region.
    pPT = [ps([128, 512], F32) for _ in range(2)]
    pO = [ps([128, QN_W], F32) for _ in range(3)]   # mm2 out (+rowsum col)
    pM = pPT[1]                       # q2cT cols 300:302, total at [0:1, 310:311]

    sems = {}
    for name in ("s_cin", "s_out", "s_qc", "pe_s", "pe_pt", "pe_o", "pt_",
                 "dve_nm", "act_p", "act_oA", "act_oD", "dve_ptr", "dve_rs",
                 "at", "dv_qc"):
        sems[name] = sem(name)
    s_cin = sems["s_cin"]; s_out = sems["s_out"]; s_qc = sems["s_qc"]
    pe_s = sems["pe_s"]; pe_pt = sems["pe_pt"]; pe_o = sems["pe_o"]
    pt_ = sems["pt_"]; dve_nm = sems["dve_nm"]; act_p = sems["act_p"]
    act_oA = sems["act_oA"]; act_oD = sems["act_oD"]
    dve_ptr = sems["dve_ptr"]; dve_rs = sems["dve_rs"]; at = sems["at"]
    dv_qc = sems["dv_qc"]

    def cin_thresh(b):
        return 96 * (b + 1) + 48

    blk = es.enter_context(nc.Block())
    with blk:
        # ---------------- SP: all DMAs ----------------
        @blk.sync
        def _(sy):
            def issue_inputs(b):
                if b >= 2:
                    # drain: all previously issued input DMAs complete so
                    # cumulative thresholds are meaningful
                    sy.wait_ge(s_cin, 96 * b + 48)
                    # WAR: batch b-2 consumers done with the b%2 buffers
                    sy.wait_ge(pe_s, 16 * (b - 1))
                    sy.wait_ge(pe_o, 16 * (b - 1))
                    sy.wait_ge(pt_, b - 1)
                sy.dma_start(ctr[b % 2][:],
                             ct_d[b].rearrange("k p c -> p k c")).then_inc(s_cin, 16)
                sy.dma_start(cbn[b % 2][:],
                             cn_d[b].rearrange("(i p) d -> p i d", p=128)).then_inc(s_cin, 16)
                sy.dma_start(qtr[b % 2][:],
                             qt_d[b].rearrange("k p q -> p k q")).then_inc(s_cin, 16)
                sy.dma_start(qnb[b % 2][:],
                             qn_d[b].rearrange("k p d -> p k d")).then_inc(s_cin, 16)
                sy.dma_start(mcs[b % 2][:], mc_d[b]).then_inc(s_cin, 16)
                sy.dma_start(mqs[b % 2][:], mq_d[b]).then_inc(s_cin, 16)

            sy.dma_start(identb[:], id_d[:]).then_inc(s_cin, 16)
            sy.dma_start(c100[:], c100_d[:]).then_inc(s_cin, 16)
            sy.dma_start(ones128[:], ones_d[:]).then_inc(s_cin, 16)
            issue_inputs(0)
            issue_inputs(1)
            for b in range(NB):
                if b + 2 < NB:
                    issue_inputs(b + 2)
                sy.wait_ge(act_oA, cnt_a(16 * (b + 1) - 1))
                sy.wait_ge(act_oD, cnt_d(16 * (b + 1) - 1))
                if b >= 2:
                    sy.wait_ge(s_out, 16 * (b - 1))
                sy.dma_start(o_d[b].rearrange("(i p) d -> p i d", p=128),
                             o_sb[b % 2][:]).then_inc(s_out, 16)
                sy.wait_ge(dv_qc, b + 1)
                sy.dma_start(qc_d[b], qc_sb[b % 2][:]).then_inc(s_qc, 16)

        # ---------------- PE ----------------
        @blk.tensor
        def _(t):
            def sim(n):
                b, i = divmod(n, NBLK)
                sl = n % 6
                if i == 0:
                    t.wait_ge(s_cin, cin_thresh(b))
                if n >= 6:
                    t.wait_ge(act_p, n - 5)   # exp(n-6) done -> pS slot free
                t.matmul(pS[:, sl, :], mcs[b % 2][:, 128 * i:128 * (i + 1)],
                         mqs[b % 2][:], start=True, stop=False)
                t.matmul(pS[:, sl, :], ctr[b % 2][:, 0, 128 * i:128 * (i + 1)],
                         qtr[b % 2][:, 0, :], start=False, stop=False)
                t.matmul(pS[:, sl, :], ctr[b % 2][:, 1, 128 * i:128 * (i + 1)],
                         qtr[b % 2][:, 1, :], start=False, stop=True).then_inc(pe_s, 1)

            def pt_tr(n):
                k = n % 2
                pb = (n // 2) % 2
                if n >= 4:
                    t.wait_ge(dve_ptr, n // 2 - 1)   # pPT[pb] prior pair copied
                ptb = pPT[pb][:].bitcast(BF16)
                tr0 = t.transpose(ptb[:, k * 256:k * 256 + 128],
                                  p_sb[n % 6][:, 0:128], identb[:])
                tr0._wait_ge(act_p, n + 1)
                t.transpose(ptb[:, k * 256 + 128:k * 256 + 256],
                            p_sb[n % 6][:, 128:256], identb[:]).then_inc(pe_pt, 1)

            def mm2(n):
                b, i = divmod(n, NBLK)
                ko = n % 3
                pp = (n // 2) % 2
                if n >= 3:
                    m = n - 3
                    t.wait_ge(act_oA, cnt_a(m))    # outcp(n-3) done
                    t.wait_ge(act_oD, cnt_d(m))
                    t.wait_ge(dve_rs, n - 2)       # recip(n-3) done
                mm0 = t.matmul(pO[ko][:], ptr[pp][:, n % 2, 0], qnb[b % 2][:, 0, :],
                               start=True, stop=False)
                mm0._wait_ge(dve_ptr, n // 2 + 1)
                t.matmul(pO[ko][:], ptr[pp][:, n % 2, 1], qnb[b % 2][:, 1, :],
                         start=False, stop=True).then_inc(pe_o, 1)

            def tail(b):
                t.wait_ge(at, b + 1)          # E_all/esum ready
                if b >= 1:
                    t.wait_ge(dv_qc, b)       # qc staging of b-1 done (pM free)
                for dh in range(2):
                    for i in range(NBLK):
                        t.matmul(pM[:, 300 + dh:301 + dh],
                                 cbn[b % 2][:, i, 128 * dh:128 * (dh + 1)],
                                 E_all[b % 2][:, i:i + 1],
                                 start=(i == 0), stop=(i == NBLK - 1))
                t.matmul(pM[0:1, 310:311], esum[b % 2][:], ones128[:],
                         start=True, stop=True).then_inc(pt_, 1)

            for g in range(NSLOT):
                n = g - L_PT
                if 0 <= n < NTOT:
                    pt_tr(n)
                n = g - L_MM
                if 0 <= n < NTOT:
                    mm2(n)
                n = g
                if 0 <= n < NTOT:
                    sim(n)
                for b in range(NB):
                    if g == 16 * b + 23:
                        tail(b)

        # ---------------- ACT ----------------
        @blk.scalar
        def _(s):
            def ex(n):
                b, i = divmod(n, NBLK)
                sl = n % 6
                if n >= 6:
                    s.wait_ge(pe_pt, n - 5)   # p_sb 6-deep WAR
                ac = s.activation(p_sb[n % 6][:], pS[:, sl, :], Exp,
                                  bias=NM[b % 2][:, i:i + 1])
                ac._wait_ge(dve_nm, 4 * b + i // 4 + 1)
                ac.then_inc(act_p, 1)

            def outcp_a(n):
                b, i = divmod(n, NBLK)
                ko = n % 3
                s.wait_ge(dve_rs, n + 1)
                if i == 0 and b >= 2:
                    s.wait_ge(s_out, 16 * (b - 1))
                s.mul(o_sb[b % 2][:, i, :], pO[ko][:, 0:256],
                      RS[b % 2][:, i:i + 1]).then_inc(act_oA, 1)

            def t1(b):
                s.wait_ge(dve_nm, 4 * (b + 1))
                if b >= 2:
                    s.wait_ge(pt_, b - 1)     # tail(b-2) done reading E/esum
                s.activation(E_all[b % 2][:], NM[b % 2][:], Exp, bias=c100[:],
                             scale=-1.0, accum_out=esum[b % 2][:]).then_inc(at, 1)

            for g in range(NSLOT):
                n = g - L_OC
                if 0 <= n < NTOT and not outcp_on_dve(n):
                    outcp_a(n)
                n = g - L_EX
                if 0 <= n < NTOT:
                    ex(n)
                for b in range(NB):
                    if g == 16 * b + 21:
                        t1(b)

        # ---------------- DVE ----------------
        @blk.vector
        def _(v):
            def nm_quad(qg):
                b, qq = divmod(qg, 4)
                if qq == 0 and b >= 2:
                    v.wait_ge(at, b - 1)   # T1(b-2) done reading NM[b%2]
                base = (4 * qg) % 6
                # pS ring is 6 slots, quads of 4 may wrap -> split reduce
                if base + 4 <= 6:
                    rd = v.tensor_reduce(NM[b % 2][:, 4 * qq:4 * qq + 4],
                                         pS[:, base:base + 4, :], AX.X, OP.max,
                                         negate=True)
                else:
                    hi = 6 - base
                    v.tensor_reduce(NM[b % 2][:, 4 * qq:4 * qq + hi],
                                    pS[:, base:6, :], AX.X, OP.max,
                                    negate=True)._wait_ge(pe_s, 4 * qg + hi)
                    rd = v.tensor_reduce(
                        NM[b % 2][:, 4 * qq + hi:4 * qq + 4],
                        pS[:, 0:4 - hi, :], AX.X, OP.max, negate=True)
                rd._wait_ge(pe_s, 4 * qg + 4)
                rd.then_inc(dve_nm, 1)

            def ptr_pair(p):
                n1 = 2 * p + 1
                if p >= 2:
                    v.wait_ge(pe_o, n1 - 3)   # mm2s of pair evicted 2 pairs ago
                cp = v.tensor_copy(ptr[p % 2][:],
                                   pPT[p % 2][:].bitcast(BF16)[:, 0:512])
                cp._wait_ge(pe_pt, n1 + 1)
                cp.then_inc(dve_ptr, 1)

            def recip(n):
                b, i = divmod(n, NBLK)
                ko = n % 3
                if i == 0 and b >= 2:
                    v.wait_ge(act_oA, cnt_a(16 * (b - 1) - 1))   # RS[b%2] WAR
                    v.wait_ge(act_oD, cnt_d(16 * (b - 1) - 1))
                rc = v.reciprocal(RS[b % 2][:, i:i + 1], pO[ko][:, 256:257])
                rc._wait_ge(pe_o, n + 1)
                rc.then_inc(dve_rs, 1)

            def outcp_d(n):
                b, i = divmod(n, NBLK)
                ko = n % 3
                v.wait_ge(dve_rs, n + 1)
                v.tensor_scalar_mul(o_sb[b % 2][:, i, :], pO[ko][:, 0:256],
                                    RS[b % 2][:, i:i + 1]).then_inc(act_oD, 1)

            def qc_stage(b):
                v.wait_ge(pt_, b + 1)
                if b >= 2:
                    v.wait_ge(s_qc, 16 * (b - 1))    # qc DMA(b-2) done
                v.tensor_copy(qc_sb[b % 2][:, 0:2], pM[:, 300:302])
                v.tensor_copy(qc_sb[b % 2][0:1, 2:3],
                              pM[0:1, 310:311]).then_inc(dv_qc, 1)

            for g in range(NSLOT):
                if g >= 5 and (g - 5) % 4 == 0 and (g - 5) // 4 < NTOT // 4:
                    nm_quad((g - 5) // 4)
                if g >= 9 and g % 2 == 1 and (g - 9) // 2 < NTOT // 2:
                    ptr_pair((g - 9) // 2)
                n = g - L_RC
                if 0 <= n < NTOT:
                    recip(n)
                n = g - L_OC
                if 0 <= n < NTOT and outcp_on_dve(n):
                    outcp_d(n)
                for b in range(NB):
                    if g == 16 * b + 25:
                        qc_stage(b)

    return nc, es


_CACHE = {}


def _get_program():
    if "nc" not in _CACHE:
        nc, es = build_program()
        _CACHE["nc"] = nc
        _CACHE["es"] = es
    return _CACHE["nc"]


def kernel(context_repr, question_repr, context_len, question_len):
    C = np.ascontiguousarray(np.asarray(context_repr, np.float32))
    Q = np.ascontiguousarray(np.asarray(question_repr, np.float32))
    context_len = np.asarray(context_len, np.int32)
    question_len = np.asarray(question_len, np.int32)
    bf16 = ml_dtypes.bfloat16

    cm = (np.arange(TC)[None, :] < context_len[:, None]).astype(np.float32)
    qm = (np.arange(TQ)[None, :] < question_len[:, None]).astype(np.float32)
    mcf = np.ascontiguousarray(np.stack([SQ * cm, np.ones_like(cm)], axis=1))
    mqf = np.ascontiguousarray(np.stack([SQ * qm, np.full_like(qm, NEG)], axis=1))

    ct = np.ascontiguousarray(C.transpose(0, 2, 1)).reshape(B, 2, 128, TC)
    cn = C.astype(bf16)
    qt = np.ascontiguousarray(Q.transpose(0, 2, 1)).reshape(B, 2, 128, TQ)
    qn = np.concatenate([Q, np.ones((B, TQ, 1), np.float32)], axis=2)
    qn = np.ascontiguousarray(qn.reshape(B, 2, 128, QN_W).astype(bf16))
    identb = np.eye(128, dtype=bf16)
    c100 = np.full((128, 1), -100.0, np.float32)
    ones128 = np.ones((128, 1), np.float32)

    nc = _get_program()
    in_maps = []
    for core in range(NCORES):
        sl = slice(core * NB, (core + 1) * NB)
        in_maps.append({
            "ct": np.ascontiguousarray(ct[sl]),
            "cn": np.ascontiguousarray(cn[sl]),
            "qt": np.ascontiguousarray(qt[sl]),
            "qn": np.ascontiguousarray(qn[sl]),
            "mcf": np.ascontiguousarray(mcf[sl]),
            "mqf": np.ascontiguousarray(mqf[sl]),
            "identb": identb,
            "c100": c100,
            "ones128": ones128,
        })

    res = run_bass_kernel_spmd(nc, in_maps, list(range(NCORES)))
    out1 = np.concatenate(
        [np.asarray(r["o"]).reshape(NB, TC, D).astype(np.float32)
         for r in res.results], axis=0)
    qc_raw = np.concatenate(
        [np.asarray(r["qc"]).reshape(NB, 128, 3) for r in res.results], axis=0)
    q2c = qc_raw[:, :, 0:2].transpose(0, 2, 1).reshape(B, D) / qc_raw[:, 0:1, 2]
    out2 = np.ascontiguousarray(np.broadcast_to(q2c[:, None, :], (B, TC, D)))
    return out1, out2


# revision 8
# speedup vs baseline: 1.7235x; 1.1521x over previous
"""BiAttention TRN2 kernel: data-parallel over batch across 8 NeuronCores.

Self-contained: hardcodes B=32, Tc=2048, Tq=256, D=256, 8 cores, 4 batches/core.

Design:
- Host pre-transposes C and Q: device receives C^T f32 (sim lhsT), C bf16
  (q2c lhsT), Q^T f32 (sim rhs), Q bf16 with a ones column (mm2 rhs).
  No C/Q transposes or PSUM->SBUF staging on PE/DVE.
- Row sums come free from the ones column of the mm2 rhs (out col 256),
  so exp needs no accumulator read.
- q2c computed as 1-row-moving matmuls (out [128,1]) -- near-zero PE cost;
  normalization by the total happens on host.
- Output stored bf16 (halves O DMA); normalize-muls split 3:1 ACT/DVE;
  row-max quad reductions on DVE; all DMAs issued from SP HWDGE queues.
- All engine threads run one continuous software-pipelined stream over the
  64 global blocks (no per-batch barriers); deep rings (pS 6 slots, p_sb 6,
  dual P^T PSUM banks, pO ring 3) keep cross-engine feedback loops slack.
"""
import numpy as np
import ml_dtypes

import concourse.bass as bass
from concourse import mybir
from concourse.bass_utils import run_bass_kernel_spmd

F32 = mybir.dt.float32
F32R = mybir.dt.float32r
BF16 = mybir.dt.bfloat16
Exp = mybir.ActivationFunctionType.Exp
AX = mybir.AxisListType
OP = mybir.AluOpType

B, TC, TQ, D = 32, 2048, 256, 256
NCORES = 8
NB = B // NCORES          # batches per core = 4
NBLK = TC // 128          # c-blocks per batch = 16
NTOT = NB * NBLK          # total blocks = 64
NEG = -(2.0 ** 96)
SQ = 2.0 ** 48
QN_W = TQ + 1             # mm2 rhs width: D cols of Q + ones column

# pipeline stage lags (in global slots)
L_EX = 5
L_PT = 7
L_MM = 10
L_RC = 12
L_OC = 13
NSLOT = NTOT + L_OC + 2


def outcp_on_dve(n):
    return n % 4 == 3


def cnt_a(m):
    """# of outcp indices 0..m handled by ACT (n%4 != 3)."""
    return (m // 4) * 3 + min(m % 4 + 1, 3)


def cnt_d(m):
    """# of outcp indices 0..m handled by DVE (n%4 == 3)."""
    return m // 4 + (1 if m % 4 == 3 else 0)


def build_program():
    nc = bass.Bass()
    ct_d = nc.declare_dram_parameter("ct", [NB, 2, 128, TC], F32R, isOutput=False)
    cn_d = nc.declare_dram_parameter("cn", [NB, TC, D], BF16, isOutput=False)
    qt_d = nc.declare_dram_parameter("qt", [NB, 2, 128, TQ], F32R, isOutput=False)
    qn_d = nc.declare_dram_parameter("qn", [NB, 2, 128, QN_W], BF16, isOutput=False)
    mc_d = nc.declare_dram_parameter("mcf", [NB, 2, TC], F32R, isOutput=False)
    mq_d = nc.declare_dram_parameter("mqf", [NB, 2, TQ], F32R, isOutput=False)
    id_d = nc.declare_dram_parameter("identb", [128, 128], BF16, isOutput=False)
    c100_d = nc.declare_dram_parameter("c100", [128, 1], F32, isOutput=False)
    ones_d = nc.declare_dram_parameter("ones128", [128, 1], F32, isOutput=False)

    o_d = nc.declare_dram_parameter("o", [NB, TC, D], BF16, isOutput=True)
    qc_d = nc.declare_dram_parameter("qc", [NB, 128, 3], F32, isOutput=True)

    from contextlib import ExitStack
    es = ExitStack()
    _ctr = [0]

    def sb(shape, dt, name=None):
        _ctr[0] += 1
        return es.enter_context(nc.sbuf_tensor(name or f"sb{_ctr[0]}", shape, dt))

    def ps(shape, dt, name=None):
        _ctr[0] += 1
        return es.enter_context(nc.psum_tensor(name or f"ps{_ctr[0]}", shape, dt))

    def sem(name):
        return es.enter_context(nc.semaphore(name))

    # ---- SBUF ----
    ctr = [sb([128, 2, TC], F32R) for _ in range(3)]    # C^T [d%128, dchunk, c]
    cbn = [sb([128, NBLK, D], BF16) for _ in range(3)]  # C natural bf16
    qtr = [sb([128, 2, TQ], F32R) for _ in range(3)]    # Q^T [d%128, dchunk, q]
    qnb = [sb([128, 2, QN_W], BF16) for _ in range(3)]  # Q nat + ones col
    mcs = [sb([2, TC], F32R) for _ in range(3)]         # mask lhsT features
    mqs = [sb([2, TQ], F32R) for _ in range(3)]         # mask rhs features
    identb = sb([128, 128], BF16)
    c100 = sb([128, 1], F32)                            # bias constant -100
    ones128 = sb([128, 1], F32)
    p_sb = [sb([128, TQ], BF16) for _ in range(6)]      # exp(S-m) (bf16), 6-deep
    ptr = [sb([128, 2, 2, 128], BF16) for _ in range(2)]  # P^T (q, blkpar, qhalf, c)
    NM = [sb([128, NBLK], F32) for _ in range(2)]       # -rowmax per block column
    RS = [sb([128, NBLK], F32) for _ in range(2)]       # 1/rowsum
    E_all = [sb([128, NBLK], BF16) for _ in range(2)]   # exp(m - 100) for q2c
    esum = [sb([128, 1], F32) for _ in range(2)]
    o_sb = [sb([128, NBLK, D], BF16) for _ in range(2)]  # output batch buffer
    qc_sb = [sb([128, 3], F32) for _ in range(2)]       # staged q2cT + total

    # ---- PSUM (8 banks) ----
    pS = ps([128, 6, 256], F32)       # sim ring, 6 slots (3 banks)
    # P^T pair banks: lower half (f32 cols 0:256) holds bf16 P^T pairs via
    # bitcast; upper half of bank 1 doubles as the q2c accumulator region.
    pPT = [ps([128, 512], F32) for _ in range(2)]
    pO = [ps([128, QN_W], F32) for _ in range(3)]   # mm2 out (+rowsum col)
    pM = pPT[1]                       # q2cT cols 300:302, total at [0:1, 310:311]

    sems = {}
    for name in ("s_cin", "s_out", "s_qc", "pe_s", "pe_pt", "pe_o", "pt_",
                 "dve_nm", "act_p", "act_oA", "act_oD", "dve_ptr", "dve_rs",
                 "at", "dv_qc"):
        sems[name] = sem(name)
    s_cin = sems["s_cin"]; s_out = sems["s_out"]; s_qc = sems["s_qc"]
    pe_s = sems["pe_s"]; pe_pt = sems["pe_pt"]; pe_o = sems["pe_o"]
    pt_ = sems["pt_"]; dve_nm = sems["dve_nm"]; act_p = sems["act_p"]
    act_oA = sems["act_oA"]; act_oD = sems["act_oD"]
    dve_ptr = sems["dve_ptr"]; dve_rs = sems["dve_rs"]; at = sems["at"]
    dv_qc = sems["dv_qc"]

    def cin_thresh(b):
        return 96 * (b + 1) + 48

    blk = es.enter_context(nc.Block())
    with blk:
        # ---------------- SP: all DMAs ----------------
        @blk.sync
        def _(sy):
            def issue_inputs(b):
                if b >= 2:
                    # drain: all previously issued input DMAs complete so
                    # cumulative thresholds are meaningful
                    sy.wait_ge(s_cin, 96 * b + 48)
                if b >= 3:
                    # WAR: batch b-3 consumers done with the b%3 buffers
                    sy.wait_ge(pe_s, 16 * (b - 2))
                    sy.wait_ge(pe_o, 16 * (b - 2))
                    sy.wait_ge(pt_, b - 2)
                sy.dma_start(ctr[b % 3][:],
                             ct_d[b].rearrange("k p c -> p k c")).then_inc(s_cin, 16)
                sy.dma_start(cbn[b % 3][:],
                             cn_d[b].rearrange("(i p) d -> p i d", p=128)).then_inc(s_cin, 16)
                sy.dma_start(qtr[b % 3][:],
                             qt_d[b].rearrange("k p q -> p k q")).then_inc(s_cin, 16)
                sy.dma_start(qnb[b % 3][:],
                             qn_d[b].rearrange("k p d -> p k d")).then_inc(s_cin, 16)
                sy.dma_start(mcs[b % 3][:], mc_d[b]).then_inc(s_cin, 16)
                sy.dma_start(mqs[b % 3][:], mq_d[b]).then_inc(s_cin, 16)

            sy.dma_start(identb[:], id_d[:]).then_inc(s_cin, 16)
            sy.dma_start(c100[:], c100_d[:]).then_inc(s_cin, 16)
            sy.dma_start(ones128[:], ones_d[:]).then_inc(s_cin, 16)
            issue_inputs(0)
            issue_inputs(1)
            for b in range(NB):
                if b + 2 < NB:
                    issue_inputs(b + 2)
                sy.wait_ge(act_oA, cnt_a(16 * (b + 1) - 1))
                sy.wait_ge(act_oD, cnt_d(16 * (b + 1) - 1))
                if b >= 2:
                    sy.wait_ge(s_out, 16 * (b - 1))
                sy.dma_start(o_d[b].rearrange("(i p) d -> p i d", p=128),
                             o_sb[b % 2][:]).then_inc(s_out, 16)
                sy.wait_ge(dv_qc, b + 1)
                sy.dma_start(qc_d[b], qc_sb[b % 2][:]).then_inc(s_qc, 16)

        # ---------------- PE ----------------
        @blk.tensor
        def _(t):
            def sim(n):
                b, i = divmod(n, NBLK)
                sl = n % 6
                if i == 0:
                    t.wait_ge(s_cin, cin_thresh(b))
                if n >= 6:
                    t.wait_ge(act_p, n - 5)   # exp(n-6) done -> pS slot free
                t.matmul(pS[:, sl, :], mcs[b % 3][:, 128 * i:128 * (i + 1)],
                         mqs[b % 3][:], start=True, stop=False)
                t.matmul(pS[:, sl, :], ctr[b % 3][:, 0, 128 * i:128 * (i + 1)],
                         qtr[b % 3][:, 0, :], start=False, stop=False)
                t.matmul(pS[:, sl, :], ctr[b % 3][:, 1, 128 * i:128 * (i + 1)],
                         qtr[b % 3][:, 1, :], start=False, stop=True).then_inc(pe_s, 1)

            def pt_tr(n):
                k = n % 2
                pb = (n // 2) % 2
                if n >= 4:
                    t.wait_ge(dve_ptr, n // 2 - 1)   # pPT[pb] prior pair copied
                ptb = pPT[pb][:].bitcast(BF16)
                tr0 = t.transpose(ptb[:, k * 256:k * 256 + 128],
                                  p_sb[n % 6][:, 0:128], identb[:])
                tr0._wait_ge(act_p, n + 1)
                t.transpose(ptb[:, k * 256 + 128:k * 256 + 256],
                            p_sb[n % 6][:, 128:256], identb[:]).then_inc(pe_pt, 1)

            def mm2(n):
                b, i = divmod(n, NBLK)
                ko = n % 3
                pp = (n // 2) % 2
                if n >= 3:
                    m = n - 3
                    t.wait_ge(act_oA, cnt_a(m))    # outcp(n-3) done
                    t.wait_ge(act_oD, cnt_d(m))
                    t.wait_ge(dve_rs, n - 2)       # recip(n-3) done
                mm0 = t.matmul(pO[ko][:], ptr[pp][:, n % 2, 0], qnb[b % 3][:, 0, :],
                               start=True, stop=False)
                mm0._wait_ge(dve_ptr, n // 2 + 1)
                t.matmul(pO[ko][:], ptr[pp][:, n % 2, 1], qnb[b % 3][:, 1, :],
                         start=False, stop=True).then_inc(pe_o, 1)

            def tail(b):
                t.wait_ge(at, b + 1)          # E_all/esum ready
                if b >= 1:
                    t.wait_ge(dv_qc, b)       # qc staging of b-1 done (pM free)
                for dh in range(2):
                    for i in range(NBLK):
                        t.matmul(pM[:, 300 + dh:301 + dh],
                                 cbn[b % 3][:, i, 128 * dh:128 * (dh + 1)],
                                 E_all[b % 2][:, i:i + 1],
                                 start=(i == 0), stop=(i == NBLK - 1))
                t.matmul(pM[0:1, 310:311], esum[b % 2][:], ones128[:],
                         start=True, stop=True).then_inc(pt_, 1)

            for g in range(NSLOT):
                n = g - L_PT
                if 0 <= n < NTOT:
                    pt_tr(n)
                n = g - L_MM
                if 0 <= n < NTOT:
                    mm2(n)
                n = g
                if 0 <= n < NTOT:
                    sim(n)
                for b in range(NB):
                    if g == 16 * b + 23:
                        tail(b)

        # ---------------- ACT ----------------
        @blk.scalar
        def _(s):
            def ex(n):
                b, i = divmod(n, NBLK)
                sl = n % 6
                if n >= 6:
                    s.wait_ge(pe_pt, n - 5)   # p_sb 6-deep WAR
                ac = s.activation(p_sb[n % 6][:], pS[:, sl, :], Exp,
                                  bias=NM[b % 2][:, i:i + 1])
                ac._wait_ge(dve_nm, 4 * b + i // 4 + 1)
                ac.then_inc(act_p, 1)

            def outcp_a(n):
                b, i = divmod(n, NBLK)
                ko = n % 3
                s.wait_ge(dve_rs, n + 1)
                if i == 0 and b >= 2:
                    s.wait_ge(s_out, 16 * (b - 1))
                s.mul(o_sb[b % 2][:, i, :], pO[ko][:, 0:256],
                      RS[b % 2][:, i:i + 1]).then_inc(act_oA, 1)

            def t1(b):
                s.wait_ge(dve_nm, 4 * (b + 1))
                if b >= 2:
                    s.wait_ge(pt_, b - 1)     # tail(b-2) done reading E/esum
                s.activation(E_all[b % 2][:], NM[b % 2][:], Exp, bias=c100[:],
                             scale=-1.0, accum_out=esum[b % 2][:]).then_inc(at, 1)

            for g in range(NSLOT):
                n = g - L_OC
                if 0 <= n < NTOT and not outcp_on_dve(n):
                    outcp_a(n)
                n = g - L_EX
                if 0 <= n < NTOT:
                    ex(n)
                for b in range(NB):
                    if g == 16 * b + 21:
                        t1(b)

        # ---------------- DVE ----------------
        @blk.vector
        def _(v):
            def nm_quad(qg):
                b, qq = divmod(qg, 4)
                if qq == 0 and b >= 2:
                    v.wait_ge(at, b - 1)   # T1(b-2) done reading NM[b%2]
                base = (4 * qg) % 6
                # pS ring is 6 slots, quads of 4 may wrap -> split reduce
                if base + 4 <= 6:
                    rd = v.tensor_reduce(NM[b % 2][:, 4 * qq:4 * qq + 4],
                                         pS[:, base:base + 4, :], AX.X, OP.max,
                                         negate=True)
                else:
                    hi = 6 - base
                    v.tensor_reduce(NM[b % 2][:, 4 * qq:4 * qq + hi],
                                    pS[:, base:6, :], AX.X, OP.max,
                                    negate=True)._wait_ge(pe_s, 4 * qg + hi)
                    rd = v.tensor_reduce(
                        NM[b % 2][:, 4 * qq + hi:4 * qq + 4],
                        pS[:, 0:4 - hi, :], AX.X, OP.max, negate=True)
                rd._wait_ge(pe_s, 4 * qg + 4)
                rd.then_inc(dve_nm, 1)

            def ptr_pair(p):
                n1 = 2 * p + 1
                if p >= 2:
                    v.wait_ge(pe_o, n1 - 3)   # mm2s of pair evicted 2 pairs ago
                cp = v.tensor_copy(ptr[p % 2][:],
                                   pPT[p % 2][:].bitcast(BF16)[:, 0:512])
                cp._wait_ge(pe_pt, n1 + 1)
                cp.then_inc(dve_ptr, 1)

            def recip(n):
                b, i = divmod(n, NBLK)
                ko = n % 3
                if i == 0 and b >= 2:
                    v.wait_ge(act_oA, cnt_a(16 * (b - 1) - 1))   # RS[b%2] WAR
                    v.wait_ge(act_oD, cnt_d(16 * (b - 1) - 1))
                rc = v.reciprocal(RS[b % 2][:, i:i + 1], pO[ko][:, 256:257])
                rc._wait_ge(pe_o, n + 1)
                rc.then_inc(dve_rs, 1)

            def outcp_d(n):
                b, i = divmod(n, NBLK)
                ko = n % 3
                v.wait_ge(dve_rs, n + 1)
                v.tensor_scalar_mul(o_sb[b % 2][:, i, :], pO[ko][:, 0:256],
                                    RS[b % 2][:, i:i + 1]).then_inc(act_oD, 1)

            def qc_stage(b):
                v.wait_ge(pt_, b + 1)
                if b >= 2:
                    v.wait_ge(s_qc, 16 * (b - 1))    # qc DMA(b-2) done
                v.tensor_copy(qc_sb[b % 2][:, 0:2], pM[:, 300:302])
                v.tensor_copy(qc_sb[b % 2][0:1, 2:3],
                              pM[0:1, 310:311]).then_inc(dv_qc, 1)

            for g in range(NSLOT):
                if g >= 5 and (g - 5) % 4 == 0 and (g - 5) // 4 < NTOT // 4:
                    nm_quad((g - 5) // 4)
                if g >= 9 and g % 2 == 1 and (g - 9) // 2 < NTOT // 2:
                    ptr_pair((g - 9) // 2)
                n = g - L_RC
                if 0 <= n < NTOT:
                    recip(n)
                n = g - L_OC
                if 0 <= n < NTOT and outcp_on_dve(n):
                    outcp_d(n)
                for b in range(NB):
                    if g == 16 * b + 25:
                        qc_stage(b)

    return nc, es


_CACHE = {}


def _get_program():
    if "nc" not in _CACHE:
        nc, es = build_program()
        _CACHE["nc"] = nc
        _CACHE["es"] = es
    return _CACHE["nc"]


def kernel(context_repr, question_repr, context_len, question_len):
    C = np.ascontiguousarray(np.asarray(context_repr, np.float32))
    Q = np.ascontiguousarray(np.asarray(question_repr, np.float32))
    context_len = np.asarray(context_len, np.int32)
    question_len = np.asarray(question_len, np.int32)
    bf16 = ml_dtypes.bfloat16

    cm = (np.arange(TC)[None, :] < context_len[:, None]).astype(np.float32)
    qm = (np.arange(TQ)[None, :] < question_len[:, None]).astype(np.float32)
    mcf = np.ascontiguousarray(np.stack([SQ * cm, np.ones_like(cm)], axis=1))
    mqf = np.ascontiguousarray(np.stack([SQ * qm, np.full_like(qm, NEG)], axis=1))

    ct = np.ascontiguousarray(C.transpose(0, 2, 1)).reshape(B, 2, 128, TC)
    cn = C.astype(bf16)
    qt = np.ascontiguousarray(Q.transpose(0, 2, 1)).reshape(B, 2, 128, TQ)
    qn = np.concatenate([Q, np.ones((B, TQ, 1), np.float32)], axis=2)
    qn = np.ascontiguousarray(qn.reshape(B, 2, 128, QN_W).astype(bf16))
    identb = np.eye(128, dtype=bf16)
    c100 = np.full((128, 1), -100.0, np.float32)
    ones128 = np.ones((128, 1), np.float32)

    nc = _get_program()
    in_maps = []
    for core in range(NCORES):
        sl = slice(core * NB, (core + 1) * NB)
        in_maps.append({
            "ct": np.ascontiguousarray(ct[sl]),
            "cn": np.ascontiguousarray(cn[sl]),
            "qt": np.ascontiguousarray(qt[sl]),
            "qn": np.ascontiguousarray(qn[sl]),
            "mcf": np.ascontiguousarray(mcf[sl]),
            "mqf": np.ascontiguousarray(mqf[sl]),
            "identb": identb,
            "c100": c100,
            "ones128": ones128,
        })

    res = run_bass_kernel_spmd(nc, in_maps, list(range(NCORES)))
    out1 = np.concatenate(
        [np.asarray(r["o"]).reshape(NB, TC, D).astype(np.float32)
         for r in res.results], axis=0)
    qc_raw = np.concatenate(
        [np.asarray(r["qc"]).reshape(NB, 128, 3) for r in res.results], axis=0)
    q2c = qc_raw[:, :, 0:2].transpose(0, 2, 1).reshape(B, D) / qc_raw[:, 0:1, 2]
    out2 = np.ascontiguousarray(np.broadcast_to(q2c[:, None, :], (B, TC, D)))
    return out1, out2


# revision 9
# speedup vs baseline: 2.0798x; 1.2067x over previous
"""BiAttention TRN2 kernel: data-parallel over batch across 8 NeuronCores.

Self-contained: hardcodes B=32, Tc=2048, Tq=256, D=256, 8 cores, 4 batches/core.

Design:
- Host pre-transposes C and Q: device receives C^T f32 (sim lhsT), C bf16
  (q2c lhsT), Q^T f32 (sim rhs), Q bf16 with a ones column (mm2 rhs).
  No C/Q transposes or PSUM->SBUF staging on PE/DVE.
- Row sums come free from the ones column of the mm2 rhs (out col 256),
  so exp needs no accumulator read.
- q2c computed as 1-row-moving matmuls (out [128,1]) -- near-zero PE cost;
  normalization by the total happens on host.
- Output stored bf16 (halves O DMA); normalize-muls split 3:1 ACT/DVE;
  row-max quad reductions on DVE; all DMAs issued from SP HWDGE queues.
- All engine threads run one continuous software-pipelined stream over the
  64 global blocks (no per-batch barriers); deep rings (pS 6 slots, p_sb 6,
  dual P^T PSUM banks, pO ring 3) keep cross-engine feedback loops slack.
"""
import numpy as np
import ml_dtypes

import concourse.bass as bass
from concourse import mybir
from concourse.bass_utils import run_bass_kernel_spmd

F32 = mybir.dt.float32
F32R = mybir.dt.float32r
BF16 = mybir.dt.bfloat16
Exp = mybir.ActivationFunctionType.Exp
AX = mybir.AxisListType
OP = mybir.AluOpType

B, TC, TQ, D = 32, 2048, 256, 256
NCORES = 8
NB = B // NCORES          # batches per core = 4
NBLK = TC // 128          # c-blocks per batch = 16
NTOT = NB * NBLK          # total blocks = 64
NEG = -(2.0 ** 96)
SQ = 2.0 ** 48
QN_W = TQ + 1             # mm2 rhs width: D cols of Q + ones column

# pipeline stage lags (in global slots)
L_EX = 5
L_PT = 7
L_MM = 10
L_RC = 12
L_OC = 13
NSLOT = NTOT + L_OC + 2


def outcp_on_dve(n):
    return n % 8 in (2, 5, 7)


def cnt_a(m):
    """# of outcp indices 0..m handled by ACT."""
    return sum(1 for j in range(m + 1) if not outcp_on_dve(j))


def cnt_d(m):
    """# of outcp indices 0..m handled by DVE."""
    return sum(1 for j in range(m + 1) if outcp_on_dve(j))


def build_program():
    nc = bass.Bass()
    ct_d = nc.declare_dram_parameter("ct", [NB, 2, 128, TC], F32R, isOutput=False)
    cn_d = nc.declare_dram_parameter("cn", [NB, TC, D], BF16, isOutput=False)
    qt_d = nc.declare_dram_parameter("qt", [NB, 2, 128, TQ], F32R, isOutput=False)
    qn_d = nc.declare_dram_parameter("qn", [NB, 2, 128, QN_W], BF16, isOutput=False)
    mc_d = nc.declare_dram_parameter("mcf", [NB, 2, TC], F32R, isOutput=False)
    mq_d = nc.declare_dram_parameter("mqf", [NB, 2, TQ], F32R, isOutput=False)
    id_d = nc.declare_dram_parameter("identb", [128, 128], BF16, isOutput=False)
    c100_d = nc.declare_dram_parameter("c100", [128, 1], F32, isOutput=False)
    ones_d = nc.declare_dram_parameter("ones128", [128, 1], F32, isOutput=False)

    o_d = nc.declare_dram_parameter("o", [NB, TC, D], BF16, isOutput=True)
    qc_d = nc.declare_dram_parameter("qc", [NB, 128, 3], F32, isOutput=True)

    from contextlib import ExitStack
    es = ExitStack()
    _ctr = [0]

    def sb(shape, dt, name=None):
        _ctr[0] += 1
        return es.enter_context(nc.sbuf_tensor(name or f"sb{_ctr[0]}", shape, dt))

    def ps(shape, dt, name=None):
        _ctr[0] += 1
        return es.enter_context(nc.psum_tensor(name or f"ps{_ctr[0]}", shape, dt))

    def sem(name):
        return es.enter_context(nc.semaphore(name))

    # ---- SBUF ----
    ctr = [sb([128, 2, TC], F32R) for _ in range(3)]    # C^T [d%128, dchunk, c]
    cbn = [sb([128, NBLK, D], BF16) for _ in range(3)]  # C natural bf16
    qtr = [sb([128, 2, TQ], F32R) for _ in range(3)]    # Q^T [d%128, dchunk, q]
    qnb = [sb([128, 2, QN_W], BF16) for _ in range(3)]  # Q nat + ones col
    mcs = [sb([2, TC], F32R) for _ in range(3)]         # mask lhsT features
    mqs = [sb([2, TQ], F32R) for _ in range(3)]         # mask rhs features
    identb = sb([128, 128], BF16)
    c100 = sb([128, 1], F32)                            # bias constant -100
    ones128 = sb([128, 1], F32)
    p_sb = [sb([128, TQ], BF16) for _ in range(6)]      # exp(S-m) (bf16), 6-deep
    ptr = [sb([128, 2, 2, 128], BF16) for _ in range(2)]  # P^T (q, blkpar, qhalf, c)
    NM = [sb([128, NBLK], F32) for _ in range(2)]       # -rowmax per block column
    RS = [sb([128, NBLK], F32) for _ in range(2)]       # 1/rowsum
    E_all = [sb([128, NBLK], BF16) for _ in range(2)]   # exp(m - 100) for q2c
    esum = [sb([128, 1], F32) for _ in range(2)]
    o_sb = [sb([128, NBLK, D], BF16) for _ in range(2)]  # output batch buffer
    qc_sb = [sb([128, 3], F32) for _ in range(2)]       # staged q2cT + total

    # ---- PSUM (8 banks) ----
    pS = ps([128, 6, 256], F32)       # sim ring, 6 slots (3 banks)
    # P^T pair banks: lower half (f32 cols 0:256) holds bf16 P^T pairs via
    # bitcast; upper half of bank 1 doubles as the q2c accumulator region.
    pPT = [ps([128, 512], F32) for _ in range(2)]
    pO = [ps([128, QN_W], F32) for _ in range(3)]   # mm2 out (+rowsum col)
    pM = pPT[1]                       # q2cT cols 300:302, total at [0:1, 310:311]

    sems = {}
    for name in ("s_cin", "s_out", "s_qc", "pe_s", "pe_pt", "pe_o", "pt_",
                 "dve_nm", "act_p", "act_oA", "act_oD", "dve_ptr", "dve_rs",
                 "at", "dv_qc"):
        sems[name] = sem(name)
    s_cin = sems["s_cin"]; s_out = sems["s_out"]; s_qc = sems["s_qc"]
    pe_s = sems["pe_s"]; pe_pt = sems["pe_pt"]; pe_o = sems["pe_o"]
    pt_ = sems["pt_"]; dve_nm = sems["dve_nm"]; act_p = sems["act_p"]
    act_oA = sems["act_oA"]; act_oD = sems["act_oD"]
    dve_ptr = sems["dve_ptr"]; dve_rs = sems["dve_rs"]; at = sems["at"]
    dv_qc = sems["dv_qc"]

    # Input DMA schedule: per batch, sim-critical tensors first, C^T in
    # 4 column-quarters so early blocks can start before the full load.
    # Consts are interleaved after batch 0's sim-critical loads.
    _sched = []
    for b in range(NB):
        _sched += [(b, "mcs"), (b, "mqs"), (b, "qtr"),
                   (b, "ct0"), (b, "ct1"), (b, "ct2"), (b, "ct3")]
        if b == 0:
            _sched += [(-1, "identb"), (-1, "c100"), (-1, "ones")]
        _sched += [(b, "qnb"), (b, "cbn")]
    _TH = {}
    for _idx, _key in enumerate(_sched):
        _TH[_key] = 16 * (_idx + 1)

    def th_sim(b, i):
        return _TH[(b, f"ct{i // 4}")]

    def th_batch_start(b):
        return _TH[(b - 1, "cbn")] if b >= 1 else 0

    blk = es.enter_context(nc.Block())
    with blk:
        # ---------------- SP: all DMAs ----------------
        @blk.sync
        def _(sy):
            def issue_one(b, tag):
                if tag == "identb":
                    return sy.dma_start(identb[:], id_d[:])
                if tag == "c100":
                    return sy.dma_start(c100[:], c100_d[:])
                if tag == "ones":
                    return sy.dma_start(ones128[:], ones_d[:])
                if tag == "mcs":
                    return sy.dma_start(mcs[b % 3][:], mc_d[b])
                if tag == "mqs":
                    return sy.dma_start(mqs[b % 3][:], mq_d[b])
                if tag == "qtr":
                    return sy.dma_start(qtr[b % 3][:],
                                        qt_d[b].rearrange("k p q -> p k q"))
                if tag.startswith("ct"):
                    q = int(tag[2])
                    return sy.dma_start(
                        ctr[b % 3][:, :, 512 * q:512 * (q + 1)],
                        ct_d[b, :, :, 512 * q:512 * (q + 1)].rearrange(
                            "k p c -> p k c"))
                if tag == "qnb":
                    return sy.dma_start(qnb[b % 3][:],
                                        qn_d[b].rearrange("k p d -> p k d"))
                if tag == "cbn":
                    return sy.dma_start(
                        cbn[b % 3][:],
                        cn_d[b].rearrange("(i p) d -> p i d", p=128))
                raise AssertionError(tag)

            def issue_inputs(b):
                if b >= 2:
                    # drain: all previously issued input DMAs complete so
                    # cumulative thresholds are meaningful
                    sy.wait_ge(s_cin, th_batch_start(b))
                if b >= 3:
                    # WAR: batch b-3 consumers done with the b%3 buffers
                    sy.wait_ge(pe_s, 16 * (b - 2))
                    sy.wait_ge(pe_o, 16 * (b - 2))
                    sy.wait_ge(pt_, b - 2)
                for bb, tag in _sched:
                    if bb == b or (b == 0 and bb == -1):
                        issue_one(b, tag).then_inc(s_cin, 16)

            issue_inputs(0)
            issue_inputs(1)
            for b in range(NB):
                if b + 2 < NB:
                    issue_inputs(b + 2)
                sy.wait_ge(act_oA, cnt_a(16 * (b + 1) - 1))
                sy.wait_ge(act_oD, cnt_d(16 * (b + 1) - 1))
                if b >= 2:
                    sy.wait_ge(s_out, 16 * (b - 1))
                sy.dma_start(o_d[b].rearrange("(i p) d -> p i d", p=128),
                             o_sb[b % 2][:]).then_inc(s_out, 16)
                sy.wait_ge(dv_qc, b + 1)
                sy.dma_start(qc_d[b], qc_sb[b % 2][:]).then_inc(s_qc, 16)

        # ---------------- PE ----------------
        @blk.tensor
        def _(t):
            def sim(n):
                b, i = divmod(n, NBLK)
                sl = n % 6
                if i % 4 == 0:
                    t.wait_ge(s_cin, th_sim(b, i))
                if n >= 6:
                    t.wait_ge(act_p, n - 5)   # exp(n-6) done -> pS slot free
                t.matmul(pS[:, sl, :], mcs[b % 3][:, 128 * i:128 * (i + 1)],
                         mqs[b % 3][:], start=True, stop=False)
                t.matmul(pS[:, sl, :], ctr[b % 3][:, 0, 128 * i:128 * (i + 1)],
                         qtr[b % 3][:, 0, :], start=False, stop=False)
                t.matmul(pS[:, sl, :], ctr[b % 3][:, 1, 128 * i:128 * (i + 1)],
                         qtr[b % 3][:, 1, :], start=False, stop=True).then_inc(pe_s, 1)

            def pt_tr(n):
                k = n % 2
                pb = (n // 2) % 2
                if n >= 4:
                    t.wait_ge(dve_ptr, n // 2 - 1)   # pPT[pb] prior pair copied
                if n == 0:
                    t.wait_ge(s_cin, _TH[(-1, "identb")])
                ptb = pPT[pb][:].bitcast(BF16)
                tr0 = t.transpose(ptb[:, k * 256:k * 256 + 128],
                                  p_sb[n % 6][:, 0:128], identb[:])
                tr0._wait_ge(act_p, n + 1)
                t.transpose(ptb[:, k * 256 + 128:k * 256 + 256],
                            p_sb[n % 6][:, 128:256], identb[:]).then_inc(pe_pt, 1)

            def mm2(n):
                b, i = divmod(n, NBLK)
                ko = n % 3
                pp = (n // 2) % 2
                if i == 0:
                    t.wait_ge(s_cin, _TH[(b, "qnb")])
                if n >= 3:
                    m = n - 3
                    t.wait_ge(act_oA, cnt_a(m))    # outcp(n-3) done
                    t.wait_ge(act_oD, cnt_d(m))
                    t.wait_ge(dve_rs, n - 2)       # recip(n-3) done
                mm0 = t.matmul(pO[ko][:], ptr[pp][:, n % 2, 0], qnb[b % 3][:, 0, :],
                               start=True, stop=False)
                mm0._wait_ge(dve_ptr, n // 2 + 1)
                t.matmul(pO[ko][:], ptr[pp][:, n % 2, 1], qnb[b % 3][:, 1, :],
                         start=False, stop=True).then_inc(pe_o, 1)

            def tail(b):
                t.wait_ge(s_cin, _TH[(b, "cbn")])
                t.wait_ge(at, b + 1)          # E_all/esum ready
                if b >= 1:
                    t.wait_ge(dv_qc, b)       # qc staging of b-1 done (pM free)
                for dh in range(2):
                    for i in range(NBLK):
                        t.matmul(pM[:, 300 + dh:301 + dh],
                                 cbn[b % 3][:, i, 128 * dh:128 * (dh + 1)],
                                 E_all[b % 2][:, i:i + 1],
                                 start=(i == 0), stop=(i == NBLK - 1))
                t.matmul(pM[0:1, 310:311], esum[b % 2][:], ones128[:],
                         start=True, stop=True).then_inc(pt_, 1)

            for g in range(NSLOT):
                n = g - L_PT
                if 0 <= n < NTOT:
                    pt_tr(n)
                n = g - L_MM
                if 0 <= n < NTOT:
                    mm2(n)
                n = g
                if 0 <= n < NTOT:
                    sim(n)
                for b in range(NB):
                    if g == 16 * b + 23:
                        tail(b)

        # ---------------- ACT ----------------
        @blk.scalar
        def _(s):
            def ex(n):
                b, i = divmod(n, NBLK)
                sl = n % 6
                if n >= 6:
                    s.wait_ge(pe_pt, n - 5)   # p_sb 6-deep WAR
                ac = s.activation(p_sb[n % 6][:], pS[:, sl, :], Exp,
                                  bias=NM[b % 2][:, i:i + 1])
                ac._wait_ge(dve_nm, 8 * b + i // 2 + 1)
                ac.then_inc(act_p, 1)

            def outcp_a(n):
                b, i = divmod(n, NBLK)
                ko = n % 3
                s.wait_ge(dve_rs, n + 1)
                if i == 0 and b >= 2:
                    s.wait_ge(s_out, 16 * (b - 1))
                s.mul(o_sb[b % 2][:, i, :], pO[ko][:, 0:256],
                      RS[b % 2][:, i:i + 1]).then_inc(act_oA, 1)

            def t1(b):
                if b == 0:
                    s.wait_ge(s_cin, _TH[(-1, "c100")])
                s.wait_ge(dve_nm, 8 * (b + 1))
                if b >= 2:
                    s.wait_ge(pt_, b - 1)     # tail(b-2) done reading E/esum
                s.activation(E_all[b % 2][:], NM[b % 2][:], Exp, bias=c100[:],
                             scale=-1.0, accum_out=esum[b % 2][:]).then_inc(at, 1)

            for g in range(NSLOT):
                n = g - L_OC
                if 0 <= n < NTOT and not outcp_on_dve(n):
                    outcp_a(n)
                n = g - L_EX
                if 0 <= n < NTOT:
                    ex(n)
                for b in range(NB):
                    if g == 16 * b + 21:
                        t1(b)

        # ---------------- DVE ----------------
        @blk.vector
        def _(v):
            def nm_pair(pg):
                b, pq = divmod(pg, 8)
                if pq == 0 and b >= 2:
                    v.wait_ge(at, b - 1)   # T1(b-2) done reading NM[b%2]
                base = (2 * pg) % 6
                rd = v.tensor_reduce(NM[b % 2][:, 2 * pq:2 * pq + 2],
                                     pS[:, base:base + 2, :], AX.X, OP.max,
                                     negate=True)
                rd._wait_ge(pe_s, 2 * pg + 2)
                rd.then_inc(dve_nm, 1)

            def ptr_pair(p):
                n1 = 2 * p + 1
                if p >= 2:
                    v.wait_ge(pe_o, n1 - 3)   # mm2s of pair evicted 2 pairs ago
                cp = v.tensor_copy(ptr[p % 2][:],
                                   pPT[p % 2][:].bitcast(BF16)[:, 0:512])
                cp._wait_ge(pe_pt, n1 + 1)
                cp.then_inc(dve_ptr, 1)

            def recip(n):
                b, i = divmod(n, NBLK)
                ko = n % 3
                if i == 0 and b >= 2:
                    v.wait_ge(act_oA, cnt_a(16 * (b - 1) - 1))   # RS[b%2] WAR
                    v.wait_ge(act_oD, cnt_d(16 * (b - 1) - 1))
                rc = v.reciprocal(RS[b % 2][:, i:i + 1], pO[ko][:, 256:257])
                rc._wait_ge(pe_o, n + 1)
                rc.then_inc(dve_rs, 1)

            def outcp_d(n):
                b, i = divmod(n, NBLK)
                ko = n % 3
                v.wait_ge(dve_rs, n + 1)
                v.tensor_scalar_mul(o_sb[b % 2][:, i, :], pO[ko][:, 0:256],
                                    RS[b % 2][:, i:i + 1]).then_inc(act_oD, 1)

            def qc_stage(b):
                v.wait_ge(pt_, b + 1)
                if b >= 2:
                    v.wait_ge(s_qc, 16 * (b - 1))    # qc DMA(b-2) done
                v.tensor_copy(qc_sb[b % 2][:, 0:2], pM[:, 300:302])
                v.tensor_copy(qc_sb[b % 2][0:1, 2:3],
                              pM[0:1, 310:311]).then_inc(dv_qc, 1)

            for g in range(NSLOT):
                if g >= 3 and (g - 3) % 2 == 0 and (g - 3) // 2 < NTOT // 2:
                    nm_pair((g - 3) // 2)
                if g >= 9 and g % 2 == 1 and (g - 9) // 2 < NTOT // 2:
                    ptr_pair((g - 9) // 2)
                n = g - L_RC
                if 0 <= n < NTOT:
                    recip(n)
                n = g - L_OC
                if 0 <= n < NTOT and outcp_on_dve(n):
                    outcp_d(n)
                for b in range(NB):
                    if g == 16 * b + 25:
                        qc_stage(b)

    return nc, es


_CACHE = {}


def _get_program():
    if "nc" not in _CACHE:
        nc, es = build_program()
        _CACHE["nc"] = nc
        _CACHE["es"] = es
    return _CACHE["nc"]


def kernel(context_repr, question_repr, context_len, question_len):
    C = np.ascontiguousarray(np.asarray(context_repr, np.float32))
    Q = np.ascontiguousarray(np.asarray(question_repr, np.float32))
    context_len = np.asarray(context_len, np.int32)
    question_len = np.asarray(question_len, np.int32)
    bf16 = ml_dtypes.bfloat16

    cm = (np.arange(TC)[None, :] < context_len[:, None]).astype(np.float32)
    qm = (np.arange(TQ)[None, :] < question_len[:, None]).astype(np.float32)
    mcf = np.ascontiguousarray(np.stack([SQ * cm, np.ones_like(cm)], axis=1))
    mqf = np.ascontiguousarray(np.stack([SQ * qm, np.full_like(qm, NEG)], axis=1))

    ct = np.ascontiguousarray(C.transpose(0, 2, 1)).reshape(B, 2, 128, TC)
    cn = C.astype(bf16)
    qt = np.ascontiguousarray(Q.transpose(0, 2, 1)).reshape(B, 2, 128, TQ)
    qn = np.concatenate([Q, np.ones((B, TQ, 1), np.float32)], axis=2)
    qn = np.ascontiguousarray(qn.reshape(B, 2, 128, QN_W).astype(bf16))
    identb = np.eye(128, dtype=bf16)
    c100 = np.full((128, 1), -100.0, np.float32)
    ones128 = np.ones((128, 1), np.float32)

    nc = _get_program()
    in_maps = []
    for core in range(NCORES):
        sl = slice(core * NB, (core + 1) * NB)
        in_maps.append({
            "ct": np.ascontiguousarray(ct[sl]),
            "cn": np.ascontiguousarray(cn[sl]),
            "qt": np.ascontiguousarray(qt[sl]),
            "qn": np.ascontiguousarray(qn[sl]),
            "mcf": np.ascontiguousarray(mcf[sl]),
            "mqf": np.ascontiguousarray(mqf[sl]),
            "identb": identb,
            "c100": c100,
            "ones128": ones128,
        })

    res = run_bass_kernel_spmd(nc, in_maps, list(range(NCORES)))
    out1 = np.concatenate(
        [np.asarray(r["o"]).reshape(NB, TC, D).astype(np.float32)
         for r in res.results], axis=0)
    qc_raw = np.concatenate(
        [np.asarray(r["qc"]).reshape(NB, 128, 3) for r in res.results], axis=0)
    q2c = qc_raw[:, :, 0:2].transpose(0, 2, 1).reshape(B, D) / qc_raw[:, 0:1, 2]
    out2 = np.ascontiguousarray(np.broadcast_to(q2c[:, None, :], (B, TC, D)))
    return out1, out2


# revision 12
# speedup vs baseline: 2.1636x; 1.0403x over previous
"""BiAttention TRN2 kernel: data-parallel over batch across 8 NeuronCores.

Self-contained: hardcodes B=32, Tc=2048, Tq=256, D=256, 8 cores, 4 batches/core.

Design:
- Host pre-transposes C and Q: device receives C^T f32 (sim lhsT), C bf16
  (q2c lhsT), Q^T f32 (sim rhs), Q bf16 with a ones column (mm2 rhs).
  No C/Q transposes or PSUM->SBUF staging on PE/DVE.
- Row sums come free from the ones column of the mm2 rhs (out col 256),
  so exp needs no accumulator read.
- q2c computed as 1-row-moving matmuls (out [128,1]) -- near-zero PE cost;
  normalization by the total happens on host.
- Output stored bf16 (halves O DMA); normalize-muls split 3:1 ACT/DVE;
  row-max quad reductions on DVE; all DMAs issued from SP HWDGE queues.
- All engine threads run one continuous software-pipelined stream over the
  64 global blocks (no per-batch barriers); deep rings (pS 6 slots, p_sb 6,
  dual P^T PSUM banks, pO ring 3) keep cross-engine feedback loops slack.
"""
import numpy as np
import ml_dtypes

import concourse.bass as bass
from concourse import mybir
from concourse.bass_utils import run_bass_kernel_spmd

F32 = mybir.dt.float32
F32R = mybir.dt.float32r
BF16 = mybir.dt.bfloat16
Exp = mybir.ActivationFunctionType.Exp
AX = mybir.AxisListType
OP = mybir.AluOpType

B, TC, TQ, D = 32, 2048, 256, 256
NCORES = 8
NB = B // NCORES          # batches per core = 4
NBLK = TC // 128          # c-blocks per batch = 16
NTOT = NB * NBLK          # total blocks = 64
NEG = -(2.0 ** 96)
SQ = 2.0 ** 48
QN_W = TQ + 1             # mm2 rhs width: D cols of Q + ones column

# pipeline stage lags (in global slots)
L_EX = 5
L_PT = 7
L_MM = 10
L_RC = 12
L_OC = 13
NSLOT = NTOT + L_OC + 2


def outcp_on_dve(n):
    return n % 8 in (2, 5, 7)


def cnt_a(m):
    """# of outcp indices 0..m handled by ACT."""
    return sum(1 for j in range(m + 1) if not outcp_on_dve(j))


def cnt_d(m):
    """# of outcp indices 0..m handled by DVE."""
    return sum(1 for j in range(m + 1) if outcp_on_dve(j))


def build_program():
    nc = bass.Bass()
    ctq_d = nc.declare_dram_parameter("ctq", [NB, 2, 128, TQ + TC], F32R,
                                      isOutput=False)
    cn_d = nc.declare_dram_parameter("cn", [NB, TC, D], BF16, isOutput=False)
    qn_d = nc.declare_dram_parameter("qn", [NB, 2, 128, QN_W], BF16, isOutput=False)
    msk_d = nc.declare_dram_parameter("msk", [NB, 2, TC + TQ], F32R, isOutput=False)
    id_d = nc.declare_dram_parameter("identb", [128, 128], BF16, isOutput=False)
    c100_d = nc.declare_dram_parameter("c100", [128, 1], F32, isOutput=False)
    ones_d = nc.declare_dram_parameter("ones128", [128, 1], F32, isOutput=False)

    o_d = nc.declare_dram_parameter("o", [NB, TC, D], BF16, isOutput=True)
    qc_d = nc.declare_dram_parameter("qc", [NB, 128, 3], F32, isOutput=True)

    from contextlib import ExitStack
    es = ExitStack()
    _ctr = [0]

    def sb(shape, dt, name=None):
        _ctr[0] += 1
        return es.enter_context(nc.sbuf_tensor(name or f"sb{_ctr[0]}", shape, dt))

    def ps(shape, dt, name=None):
        _ctr[0] += 1
        return es.enter_context(nc.psum_tensor(name or f"ps{_ctr[0]}", shape, dt))

    def sem(name):
        return es.enter_context(nc.semaphore(name))

    # ---- SBUF ----
    # merged [Q^T | C^T] per batch: cols 0:TQ = Q^T, TQ: = C^T
    ctq = [sb([128, 2, TQ + TC], F32R) for _ in range(3)]
    cbn = [sb([128, NBLK, D], BF16) for _ in range(3)]  # C natural bf16
    qnb = [sb([128, 2, QN_W], BF16) for _ in range(3)]  # Q nat + ones col
    msk = [sb([2, TC + TQ], F32R) for _ in range(3)]    # [c-mask | q-mask] features
    identb = sb([128, 128], BF16)
    c100 = sb([128, 1], F32)                            # bias constant -100
    ones128 = sb([128, 1], F32)
    p_sb = [sb([128, TQ], BF16) for _ in range(6)]      # exp(S-m) (bf16), 6-deep
    ptr = [sb([128, 2, 2, 128], BF16) for _ in range(2)]  # P^T (q, blkpar, qhalf, c)
    NM = [sb([128, NBLK], F32) for _ in range(2)]       # -rowmax per block column
    RS = [sb([128, NBLK], F32) for _ in range(2)]       # 1/rowsum
    E_all = [sb([128, NBLK], BF16) for _ in range(2)]   # exp(m - 100) for q2c
    esum = [sb([128, 1], F32) for _ in range(2)]
    o_sb = [sb([128, NBLK, D], BF16) for _ in range(2)]  # output batch buffer
    qc_sb = [sb([128, 3], F32) for _ in range(2)]       # staged q2cT + total

    # ---- PSUM (8 banks) ----
    pS = ps([128, 6, 256], F32)       # sim ring, 6 slots (3 banks)
    # P^T pair banks: lower half (f32 cols 0:256) holds bf16 P^T pairs via
    # bitcast; upper half of bank 1 doubles as the q2c accumulator region.
    pPT = [ps([128, 512], F32) for _ in range(2)]
    pO = [ps([128, QN_W], F32) for _ in range(3)]   # mm2 out (+rowsum col)
    pM = pPT[1]                       # q2cT cols 300:302, total at [0:1, 310:311]

    sems = {}
    for name in ("s_cin", "s_out", "s_qc", "pe_s", "pe_pt", "pe_o", "pt_",
                 "dve_nm", "act_p", "act_oA", "act_oD", "dve_ptr", "dve_rs",
                 "at", "dv_qc"):
        sems[name] = sem(name)
    s_cin = sems["s_cin"]; s_out = sems["s_out"]; s_qc = sems["s_qc"]
    pe_s = sems["pe_s"]; pe_pt = sems["pe_pt"]; pe_o = sems["pe_o"]
    pt_ = sems["pt_"]; dve_nm = sems["dve_nm"]; act_p = sems["act_p"]
    act_oA = sems["act_oA"]; act_oD = sems["act_oD"]
    dve_ptr = sems["dve_ptr"]; dve_rs = sems["dve_rs"]; at = sems["at"]
    dv_qc = sems["dv_qc"]

    # Input DMA schedule: per batch, sim-critical tensors first, C^T in
    # 4 column-quarters so early blocks can start before the full load.
    # Consts are interleaved after batch 0's sim-critical loads.
    _sched = []
    for b in range(NB):
        _sched += [(b, "msk"),
                   (b, "ctq0"), (b, "ctq1"), (b, "ctq2"), (b, "ctq3")]
        if b == 0:
            _sched += [(-1, "identb"), (-1, "c100"), (-1, "ones")]
        _sched += [(b, "qnb"), (b, "cbn")]
    _TH = {}
    for _idx, _key in enumerate(_sched):
        _TH[_key] = 16 * (_idx + 1)

    def th_sim(b, i):
        return _TH[(b, f"ctq{i // 4}")]

    def th_batch_start(b):
        return _TH[(b - 1, "cbn")] if b >= 1 else 0

    blk = es.enter_context(nc.Block())
    with blk:
        # ---------------- SP: all DMAs ----------------
        @blk.sync
        def _(sy):
            def issue_one(b, tag):
                if tag == "identb":
                    return sy.dma_start(identb[:], id_d[:])
                if tag == "c100":
                    return sy.dma_start(c100[:], c100_d[:])
                if tag == "ones":
                    return sy.dma_start(ones128[:], ones_d[:])
                if tag == "msk":
                    return sy.dma_start(msk[b % 3][:], msk_d[b])
                if tag.startswith("ctq"):
                    q = int(tag[3])
                    lo = 0 if q == 0 else TQ + 512 * q
                    hi = TQ + 512 * (q + 1)
                    return sy.dma_start(
                        ctq[b % 3][:, :, lo:hi],
                        ctq_d[b, :, :, lo:hi].rearrange("k p c -> p k c"))
                if tag == "qnb":
                    return sy.dma_start(qnb[b % 3][:],
                                        qn_d[b].rearrange("k p d -> p k d"))
                if tag == "cbn":
                    return sy.dma_start(
                        cbn[b % 3][:],
                        cn_d[b].rearrange("(i p) d -> p i d", p=128))
                raise AssertionError(tag)

            def issue_inputs(b):
                if b >= 2:
                    # drain: all previously issued input DMAs complete so
                    # cumulative thresholds are meaningful
                    sy.wait_ge(s_cin, th_batch_start(b))
                if b >= 3:
                    # WAR: batch b-3 consumers done with the b%3 buffers
                    sy.wait_ge(pe_s, 16 * (b - 2))
                    sy.wait_ge(pe_o, 16 * (b - 2))
                    sy.wait_ge(pt_, b - 2)
                for bb, tag in _sched:
                    if bb == b or (b == 0 and bb == -1):
                        issue_one(b, tag).then_inc(s_cin, 16)

            issue_inputs(0)
            issue_inputs(1)
            for b in range(NB):
                if b + 2 < NB:
                    issue_inputs(b + 2)
                if b >= 2:
                    sy.wait_ge(s_out, 64 * (b - 1))
                for q4 in range(4):
                    m = 16 * b + 4 * q4 + 3
                    sy.wait_ge(act_oA, cnt_a(m))
                    sy.wait_ge(act_oD, cnt_d(m))
                    sy.dma_start(
                        o_d[b, 512 * q4:512 * (q4 + 1)].rearrange(
                            "(i p) d -> p i d", p=128),
                        o_sb[b % 2][:, 4 * q4:4 * (q4 + 1), :]).then_inc(s_out, 16)
                sy.wait_ge(dv_qc, b + 1)
                sy.dma_start(qc_d[b], qc_sb[b % 2][:]).then_inc(s_qc, 16)

        # ---------------- PE ----------------
        @blk.tensor
        def _(t):
            def sim(n):
                b, i = divmod(n, NBLK)
                sl = n % 6
                if i % 4 == 0:
                    t.wait_ge(s_cin, th_sim(b, i))
                if n >= 6:
                    t.wait_ge(act_p, n - 5)   # exp(n-6) done -> pS slot free
                t.matmul(pS[:, sl, :],
                         msk[b % 3][:, TQ + 128 * i:TQ + 128 * (i + 1)],
                         msk[b % 3][:, 0:TQ], start=True, stop=False)
                t.matmul(pS[:, sl, :],
                         ctq[b % 3][:, 0, TQ + 128 * i:TQ + 128 * (i + 1)],
                         ctq[b % 3][:, 0, 0:TQ], start=False, stop=False)
                t.matmul(pS[:, sl, :],
                         ctq[b % 3][:, 1, TQ + 128 * i:TQ + 128 * (i + 1)],
                         ctq[b % 3][:, 1, 0:TQ], start=False,
                         stop=True).then_inc(pe_s, 1)

            def pt_tr(n):
                k = n % 2
                pb = (n // 2) % 2
                if n >= 4:
                    t.wait_ge(dve_ptr, n // 2 - 1)   # pPT[pb] prior pair copied
                if n == 0:
                    t.wait_ge(s_cin, _TH[(-1, "identb")])
                ptb = pPT[pb][:].bitcast(BF16)
                tr0 = t.transpose(ptb[:, k * 256:k * 256 + 128],
                                  p_sb[n % 6][:, 0:128], identb[:])
                tr0._wait_ge(act_p, n + 1)
                t.transpose(ptb[:, k * 256 + 128:k * 256 + 256],
                            p_sb[n % 6][:, 128:256], identb[:]).then_inc(pe_pt, 1)

            def mm2(n):
                b, i = divmod(n, NBLK)
                ko = n % 3
                pp = (n // 2) % 2
                if i == 0:
                    t.wait_ge(s_cin, _TH[(b, "qnb")])
                if n >= 3:
                    m = n - 3
                    t.wait_ge(act_oA, cnt_a(m))    # outcp(n-3) done
                    t.wait_ge(act_oD, cnt_d(m))
                    t.wait_ge(dve_rs, n - 2)       # recip(n-3) done
                mm0 = t.matmul(pO[ko][:], ptr[pp][:, n % 2, 0], qnb[b % 3][:, 0, :],
                               start=True, stop=False)
                mm0._wait_ge(dve_ptr, n // 2 + 1)
                t.matmul(pO[ko][:], ptr[pp][:, n % 2, 1], qnb[b % 3][:, 1, :],
                         start=False, stop=True).then_inc(pe_o, 1)

            def tail(b):
                t.wait_ge(s_cin, _TH[(b, "cbn")])
                t.wait_ge(at, b + 1)          # E_all/esum ready
                if b >= 1:
                    t.wait_ge(dv_qc, b)       # qc staging of b-1 done (pM free)
                for dh in range(2):
                    for i in range(NBLK):
                        t.matmul(pM[:, 300 + dh:301 + dh],
                                 cbn[b % 3][:, i, 128 * dh:128 * (dh + 1)],
                                 E_all[b % 2][:, i:i + 1],
                                 start=(i == 0), stop=(i == NBLK - 1))
                t.matmul(pM[0:1, 310:311], esum[b % 2][:], ones128[:],
                         start=True, stop=True).then_inc(pt_, 1)

            for g in range(NSLOT):
                n = g - L_PT
                if 0 <= n < NTOT:
                    pt_tr(n)
                n = g - L_MM
                if 0 <= n < NTOT:
                    mm2(n)
                n = g
                if 0 <= n < NTOT:
                    sim(n)
                for b in range(NB):
                    if g == 16 * b + 23:
                        tail(b)

        # ---------------- ACT ----------------
        @blk.scalar
        def _(s):
            def ex(n):
                b, i = divmod(n, NBLK)
                sl = n % 6
                if n >= 6:
                    s.wait_ge(pe_pt, n - 5)   # p_sb 6-deep WAR
                ac = s.activation(p_sb[n % 6][:], pS[:, sl, :], Exp,
                                  bias=NM[b % 2][:, i:i + 1])
                ac._wait_ge(dve_nm, 8 * b + i // 2 + 1)
                ac.then_inc(act_p, 1)

            def outcp_a(n):
                b, i = divmod(n, NBLK)
                ko = n % 3
                s.wait_ge(dve_rs, n + 1)
                if i == 0 and b >= 2:
                    s.wait_ge(s_out, 64 * (b - 1))
                s.mul(o_sb[b % 2][:, i, :], pO[ko][:, 0:256],
                      RS[b % 2][:, i:i + 1]).then_inc(act_oA, 1)

            def t1(b):
                if b == 0:
                    s.wait_ge(s_cin, _TH[(-1, "c100")])
                s.wait_ge(dve_nm, 8 * (b + 1))
                if b >= 2:
                    s.wait_ge(pt_, b - 1)     # tail(b-2) done reading E/esum
                s.activation(E_all[b % 2][:], NM[b % 2][:], Exp, bias=c100[:],
                             scale=-1.0, accum_out=esum[b % 2][:]).then_inc(at, 1)

            for g in range(NSLOT):
                n = g - L_OC
                if 0 <= n < NTOT and not outcp_on_dve(n):
                    outcp_a(n)
                n = g - L_EX
                if 0 <= n < NTOT:
                    ex(n)
                for b in range(NB):
                    if g == 16 * b + 21:
                        t1(b)

        # ---------------- DVE ----------------
        @blk.vector
        def _(v):
            def nm_pair(pg):
                b, pq = divmod(pg, 8)
                if pq == 0 and b >= 2:
                    v.wait_ge(at, b - 1)   # T1(b-2) done reading NM[b%2]
                base = (2 * pg) % 6
                rd = v.tensor_reduce(NM[b % 2][:, 2 * pq:2 * pq + 2],
                                     pS[:, base:base + 2, :], AX.X, OP.max,
                                     negate=True)
                rd._wait_ge(pe_s, 2 * pg + 2)
                rd.then_inc(dve_nm, 1)

            def ptr_pair(p):
                n1 = 2 * p + 1
                if p >= 2:
                    v.wait_ge(pe_o, n1 - 3)   # mm2s of pair evicted 2 pairs ago
                cp = v.tensor_copy(ptr[p % 2][:],
                                   pPT[p % 2][:].bitcast(BF16)[:, 0:512])
                cp._wait_ge(pe_pt, n1 + 1)
                cp.then_inc(dve_ptr, 1)

            def recip(n):
                b, i = divmod(n, NBLK)
                ko = n % 3
                if i == 0 and b >= 2:
                    v.wait_ge(act_oA, cnt_a(16 * (b - 1) - 1))   # RS[b%2] WAR
                    v.wait_ge(act_oD, cnt_d(16 * (b - 1) - 1))
                rc = v.reciprocal(RS[b % 2][:, i:i + 1], pO[ko][:, 256:257])
                rc._wait_ge(pe_o, n + 1)
                rc.then_inc(dve_rs, 1)

            def outcp_d(n):
                b, i = divmod(n, NBLK)
                ko = n % 3
                v.wait_ge(dve_rs, n + 1)
                v.tensor_scalar_mul(o_sb[b % 2][:, i, :], pO[ko][:, 0:256],
                                    RS[b % 2][:, i:i + 1]).then_inc(act_oD, 1)

            def qc_stage(b):
                v.wait_ge(pt_, b + 1)
                if b >= 2:
                    v.wait_ge(s_qc, 16 * (b - 1))    # qc DMA(b-2) done
                v.tensor_copy(qc_sb[b % 2][:, 0:2], pM[:, 300:302])
                v.tensor_copy(qc_sb[b % 2][0:1, 2:3],
                              pM[0:1, 310:311]).then_inc(dv_qc, 1)

            for g in range(NSLOT):
                if g >= 3 and (g - 3) % 2 == 0 and (g - 3) // 2 < NTOT // 2:
                    nm_pair((g - 3) // 2)
                if g >= 9 and g % 2 == 1 and (g - 9) // 2 < NTOT // 2:
                    ptr_pair((g - 9) // 2)
                n = g - L_RC
                if 0 <= n < NTOT:
                    recip(n)
                n = g - L_OC
                if 0 <= n < NTOT and outcp_on_dve(n):
                    outcp_d(n)
                for b in range(NB):
                    if g == 16 * b + 25:
                        qc_stage(b)

    return nc, es


_CACHE = {}


def _get_program():
    if "nc" not in _CACHE:
        nc, es = build_program()
        _CACHE["nc"] = nc
        _CACHE["es"] = es
    return _CACHE["nc"]


def kernel(context_repr, question_repr, context_len, question_len):
    C = np.ascontiguousarray(np.asarray(context_repr, np.float32))
    Q = np.ascontiguousarray(np.asarray(question_repr, np.float32))
    context_len = np.asarray(context_len, np.int32)
    question_len = np.asarray(question_len, np.int32)
    bf16 = ml_dtypes.bfloat16

    cm = (np.arange(TC)[None, :] < context_len[:, None]).astype(np.float32)
    qm = (np.arange(TQ)[None, :] < question_len[:, None]).astype(np.float32)
    mcf = np.stack([SQ * cm, np.ones_like(cm)], axis=1)
    mqf = np.stack([SQ * qm, np.full_like(qm, NEG)], axis=1)
    mskh = np.ascontiguousarray(np.concatenate([mqf, mcf], axis=2))

    ct = C.transpose(0, 2, 1).reshape(B, 2, 128, TC)
    qt = Q.transpose(0, 2, 1).reshape(B, 2, 128, TQ)
    ctq = np.ascontiguousarray(np.concatenate([qt, ct], axis=3))
    cn = C.astype(bf16)
    qn = np.concatenate([Q, np.ones((B, TQ, 1), np.float32)], axis=2)
    qn = np.ascontiguousarray(qn.reshape(B, 2, 128, QN_W).astype(bf16))
    identb = np.eye(128, dtype=bf16)
    c100 = np.full((128, 1), -100.0, np.float32)
    ones128 = np.ones((128, 1), np.float32)

    nc = _get_program()
    in_maps = []
    for core in range(NCORES):
        sl = slice(core * NB, (core + 1) * NB)
        in_maps.append({
            "ctq": np.ascontiguousarray(ctq[sl]),
            "cn": np.ascontiguousarray(cn[sl]),
            "qn": np.ascontiguousarray(qn[sl]),
            "msk": np.ascontiguousarray(mskh[sl]),
            "identb": identb,
            "c100": c100,
            "ones128": ones128,
        })

    res = run_bass_kernel_spmd(nc, in_maps, list(range(NCORES)))
    out1 = np.concatenate(
        [np.asarray(r["o"]).reshape(NB, TC, D).astype(np.float32)
         for r in res.results], axis=0)
    qc_raw = np.concatenate(
        [np.asarray(r["qc"]).reshape(NB, 128, 3) for r in res.results], axis=0)
    q2c = qc_raw[:, :, 0:2].transpose(0, 2, 1).reshape(B, D) / qc_raw[:, 0:1, 2]
    out2 = np.ascontiguousarray(np.broadcast_to(q2c[:, None, :], (B, TC, D)))
    return out1, out2


# revision 14
# speedup vs baseline: 2.2187x; 1.0255x over previous
"""BiAttention TRN2 kernel: data-parallel over batch across 8 NeuronCores.

Self-contained: hardcodes B=32, Tc=2048, Tq=256, D=256, 8 cores, 4 batches/core.

Design:
- Host pre-transposes C and Q: device receives C^T f32 (sim lhsT), C bf16
  (q2c lhsT), Q^T f32 (sim rhs), Q bf16 with a ones column (mm2 rhs).
  No C/Q transposes or PSUM->SBUF staging on PE/DVE.
- Row sums come free from the ones column of the mm2 rhs (out col 256),
  so exp needs no accumulator read.
- q2c computed as 1-row-moving matmuls (out [128,1]) -- near-zero PE cost;
  normalization by the total happens on host.
- Output stored bf16 (halves O DMA); normalize-muls split 3:1 ACT/DVE;
  row-max quad reductions on DVE; all DMAs issued from SP HWDGE queues.
- All engine threads run one continuous software-pipelined stream over the
  64 global blocks (no per-batch barriers); deep rings (pS 6 slots, p_sb 6,
  dual P^T PSUM banks, pO ring 3) keep cross-engine feedback loops slack.
"""
import numpy as np
import ml_dtypes

import concourse.bass as bass
from concourse import mybir
from concourse.bass_utils import run_bass_kernel_spmd

F32 = mybir.dt.float32
F32R = mybir.dt.float32r
BF16 = mybir.dt.bfloat16
Exp = mybir.ActivationFunctionType.Exp
AX = mybir.AxisListType
OP = mybir.AluOpType

B, TC, TQ, D = 32, 2048, 256, 256
NCORES = 8
NB = B // NCORES          # batches per core = 4
NBLK = TC // 128          # c-blocks per batch = 16
NTOT = NB * NBLK          # total blocks = 64
NEG = -(2.0 ** 96)
SQ = 2.0 ** 48
QN_W = TQ + 1             # mm2 rhs width: D cols of Q + ones column

# pipeline stage lags (in global slots)
L_EX = 5
L_PT = 7
L_MM = 10
L_RC = 12
L_OC = 13
NSLOT = NTOT + L_OC + 2


def outcp_on_dve(n):
    return n % 8 in (2, 5, 7)


def cnt_a(m):
    """# of outcp indices 0..m handled by ACT."""
    return sum(1 for j in range(m + 1) if not outcp_on_dve(j))


def cnt_d(m):
    """# of outcp indices 0..m handled by DVE."""
    return sum(1 for j in range(m + 1) if outcp_on_dve(j))


def build_program():
    nc = bass.Bass()
    ctq_d = nc.declare_dram_parameter("ctq", [NB, 2, 128, TQ + TC], F32R,
                                      isOutput=False)
    cn_d = nc.declare_dram_parameter("cn", [NB, TC, D], BF16, isOutput=False)
    qn_d = nc.declare_dram_parameter("qn", [NB, 2, 128, QN_W], BF16, isOutput=False)
    msk_d = nc.declare_dram_parameter("msk", [NB, 2, TC + TQ], F32R, isOutput=False)
    id_d = nc.declare_dram_parameter("identb", [128, 128], BF16, isOutput=False)
    c100_d = nc.declare_dram_parameter("c100", [128, 1], F32, isOutput=False)
    ones_d = nc.declare_dram_parameter("ones128", [128, 1], F32, isOutput=False)

    o_d = nc.declare_dram_parameter("o", [NB, TC, D], BF16, isOutput=True)
    qc_d = nc.declare_dram_parameter("qc", [NB, 128, 3], F32, isOutput=True)

    from contextlib import ExitStack
    es = ExitStack()
    _ctr = [0]

    def sb(shape, dt, name=None):
        _ctr[0] += 1
        return es.enter_context(nc.sbuf_tensor(name or f"sb{_ctr[0]}", shape, dt))

    def ps(shape, dt, name=None):
        _ctr[0] += 1
        return es.enter_context(nc.psum_tensor(name or f"ps{_ctr[0]}", shape, dt))

    def sem(name):
        return es.enter_context(nc.semaphore(name))

    # ---- SBUF ----
    # merged [Q^T | C^T] per batch: cols 0:TQ = Q^T, TQ: = C^T
    ctq = [sb([128, 2, TQ + TC], F32R) for _ in range(3)]
    cbn = [sb([128, NBLK, D], BF16) for _ in range(3)]  # C natural bf16
    qnb = [sb([128, 2, QN_W], BF16) for _ in range(3)]  # Q nat + ones col
    msk = [sb([2, TC + TQ], F32R) for _ in range(3)]    # [c-mask | q-mask] features
    identb = sb([128, 128], BF16)
    c100 = sb([128, 1], F32)                            # bias constant -100
    ones128 = sb([128, 1], F32)
    p_sb = [sb([128, TQ], BF16) for _ in range(6)]      # exp(S-m) (bf16), 6-deep
    ptr = [sb([128, 2, 2, 128], BF16) for _ in range(2)]  # P^T (q, blkpar, qhalf, c)
    NM = [sb([128, NBLK], F32) for _ in range(2)]       # -rowmax per block column
    RS = [sb([128, NBLK], F32) for _ in range(2)]       # 1/rowsum
    E_all = [sb([128, NBLK], BF16) for _ in range(2)]   # exp(m - 100) for q2c
    esum = [sb([128, 1], F32) for _ in range(2)]
    o_sb = [sb([128, NBLK, D], BF16) for _ in range(2)]  # output batch buffer
    qc_sb = [sb([128, 3], F32) for _ in range(2)]       # staged q2cT + total

    # ---- PSUM (8 banks) ----
    pS = ps([128, 6, 256], F32)       # sim ring, 6 slots (3 banks)
    # P^T pair banks: lower half (f32 cols 0:256) holds bf16 P^T pairs via
    # bitcast; upper half of bank 1 doubles as the q2c accumulator region.
    pPT = [ps([128, 512], F32) for _ in range(2)]
    pO = [ps([128, QN_W], F32) for _ in range(3)]   # mm2 out (+rowsum col)
    pM = pPT[1]                       # q2cT cols 300:302, total at [0:1, 310:311]

    sems = {}
    for name in ("s_cin", "s_out", "s_qc", "pe_s", "pe_pt", "pe_o", "pt_",
                 "dve_nm", "act_p", "act_oA", "act_oD", "dve_ptr", "dve_rs",
                 "at", "dv_qc"):
        sems[name] = sem(name)
    s_cin = sems["s_cin"]; s_out = sems["s_out"]; s_qc = sems["s_qc"]
    pe_s = sems["pe_s"]; pe_pt = sems["pe_pt"]; pe_o = sems["pe_o"]
    pt_ = sems["pt_"]; dve_nm = sems["dve_nm"]; act_p = sems["act_p"]
    act_oA = sems["act_oA"]; act_oD = sems["act_oD"]
    dve_ptr = sems["dve_ptr"]; dve_rs = sems["dve_rs"]; at = sems["at"]
    dv_qc = sems["dv_qc"]

    # Input DMA schedule: per batch, sim-critical tensors first, C^T in
    # 4 column-quarters so early blocks can start before the full load.
    # Consts are interleaved after batch 0's sim-critical loads.
    CTQ_CUTS = [0, TQ + 128, TQ + 128 * 5, TQ + 128 * 9, TQ + 128 * 13,
                TQ + TC]
    NCHUNK = len(CTQ_CUTS) - 1
    TH_I = {0: 0, 1: 1, 5: 2, 9: 3, 13: 4}   # block -> chunk it needs
    _sched = []
    for b in range(NB):
        _sched += [(b, "msk")]
        _sched += [(b, f"ctq{q}") for q in range(NCHUNK)]
        if b == 0:
            _sched += [(-1, "identb"), (-1, "c100"), (-1, "ones")]
        _sched += [(b, "qnb"), (b, "cbn")]
    _TH = {}
    for _idx, _key in enumerate(_sched):
        _TH[_key] = 16 * (_idx + 1)

    def th_sim(b, i):
        return _TH[(b, f"ctq{TH_I[i]}")]

    def th_batch_start(b):
        return _TH[(b - 1, "cbn")] if b >= 1 else 0

    blk = es.enter_context(nc.Block())
    with blk:
        # ---------------- SP: all DMAs ----------------
        @blk.sync
        def _(sy):
            def issue_one(b, tag):
                if tag == "identb":
                    return sy.dma_start(identb[:], id_d[:])
                if tag == "c100":
                    return sy.dma_start(c100[:], c100_d[:])
                if tag == "ones":
                    return sy.dma_start(ones128[:], ones_d[:])
                if tag == "msk":
                    return sy.dma_start(msk[b % 3][:], msk_d[b])
                if tag.startswith("ctq"):
                    q = int(tag[3])
                    lo, hi = CTQ_CUTS[q], CTQ_CUTS[q + 1]
                    return sy.dma_start(
                        ctq[b % 3][:, :, lo:hi],
                        ctq_d[b, :, :, lo:hi].rearrange("k p c -> p k c"))
                if tag == "qnb":
                    return sy.dma_start(qnb[b % 3][:],
                                        qn_d[b].rearrange("k p d -> p k d"))
                if tag == "cbn":
                    return sy.dma_start(
                        cbn[b % 3][:],
                        cn_d[b].rearrange("(i p) d -> p i d", p=128))
                raise AssertionError(tag)

            def issue_inputs(b):
                if b >= 2:
                    # drain: all previously issued input DMAs complete so
                    # cumulative thresholds are meaningful
                    sy.wait_ge(s_cin, th_batch_start(b))
                if b >= 3:
                    # WAR: batch b-3 consumers done with the b%3 buffers
                    sy.wait_ge(pe_s, 16 * (b - 2))
                    sy.wait_ge(pe_o, 16 * (b - 2))
                    sy.wait_ge(pt_, b - 2)
                for bb, tag in _sched:
                    if bb == b or (b == 0 and bb == -1):
                        issue_one(b, tag).then_inc(s_cin, 16)

            issue_inputs(0)
            issue_inputs(1)
            for b in range(NB):
                if b + 2 < NB:
                    issue_inputs(b + 2)
                if b >= 2:
                    sy.wait_ge(s_out, 64 * (b - 1))
                for q4 in range(4):
                    m = 16 * b + 4 * q4 + 3
                    sy.wait_ge(act_oA, cnt_a(m))
                    sy.wait_ge(act_oD, cnt_d(m))
                    sy.dma_start(
                        o_d[b, 512 * q4:512 * (q4 + 1)].rearrange(
                            "(i p) d -> p i d", p=128),
                        o_sb[b % 2][:, 4 * q4:4 * (q4 + 1), :]).then_inc(s_out, 16)
                sy.wait_ge(dv_qc, b + 1)
                sy.dma_start(qc_d[b], qc_sb[b % 2][:]).then_inc(s_qc, 16)

        # ---------------- PE ----------------
        @blk.tensor
        def _(t):
            def sim(n):
                b, i = divmod(n, NBLK)
                sl = n % 6
                if i in TH_I:
                    t.wait_ge(s_cin, th_sim(b, i))
                if n >= 6:
                    t.wait_ge(act_p, n - 5)   # exp(n-6) done -> pS slot free
                t.matmul(pS[:, sl, :],
                         msk[b % 3][:, TQ + 128 * i:TQ + 128 * (i + 1)],
                         msk[b % 3][:, 0:TQ], start=True, stop=False)
                t.matmul(pS[:, sl, :],
                         ctq[b % 3][:, 0, TQ + 128 * i:TQ + 128 * (i + 1)],
                         ctq[b % 3][:, 0, 0:TQ], start=False, stop=False)
                t.matmul(pS[:, sl, :],
                         ctq[b % 3][:, 1, TQ + 128 * i:TQ + 128 * (i + 1)],
                         ctq[b % 3][:, 1, 0:TQ], start=False,
                         stop=True).then_inc(pe_s, 1)

            def pt_tr(n):
                k = n % 2
                pb = (n // 2) % 2
                if n >= 4:
                    t.wait_ge(dve_ptr, n // 2 - 1)   # pPT[pb] prior pair copied
                if n == 0:
                    t.wait_ge(s_cin, _TH[(-1, "identb")])
                ptb = pPT[pb][:].bitcast(BF16)
                tr0 = t.transpose(ptb[:, k * 256:k * 256 + 128],
                                  p_sb[n % 6][:, 0:128], identb[:])
                tr0._wait_ge(act_p, n + 1)
                t.transpose(ptb[:, k * 256 + 128:k * 256 + 256],
                            p_sb[n % 6][:, 128:256], identb[:]).then_inc(pe_pt, 1)

            def mm2(n):
                b, i = divmod(n, NBLK)
                ko = n % 3
                pp = (n // 2) % 2
                if i == 0:
                    t.wait_ge(s_cin, _TH[(b, "qnb")])
                if n >= 3:
                    m = n - 3
                    t.wait_ge(act_oA, cnt_a(m))    # outcp(n-3) done
                    t.wait_ge(act_oD, cnt_d(m))
                    t.wait_ge(dve_rs, n - 2)       # recip(n-3) done
                mm0 = t.matmul(pO[ko][:], ptr[pp][:, n % 2, 0], qnb[b % 3][:, 0, :],
                               start=True, stop=False)
                mm0._wait_ge(dve_ptr, n // 2 + 1)
                t.matmul(pO[ko][:], ptr[pp][:, n % 2, 1], qnb[b % 3][:, 1, :],
                         start=False, stop=True).then_inc(pe_o, 1)

            def tail(b):
                t.wait_ge(s_cin, _TH[(b, "cbn")])
                t.wait_ge(at, b + 1)          # E_all/esum ready
                if b >= 1:
                    t.wait_ge(dv_qc, b)       # qc staging of b-1 done (pM free)
                for dh in range(2):
                    for i in range(NBLK):
                        t.matmul(pM[:, 300 + dh:301 + dh],
                                 cbn[b % 3][:, i, 128 * dh:128 * (dh + 1)],
                                 E_all[b % 2][:, i:i + 1],
                                 start=(i == 0), stop=(i == NBLK - 1))
                t.matmul(pM[0:1, 310:311], esum[b % 2][:], ones128[:],
                         start=True, stop=True).then_inc(pt_, 1)

            for g in range(NSLOT):
                n = g - L_PT
                if 0 <= n < NTOT:
                    pt_tr(n)
                n = g - L_MM
                if 0 <= n < NTOT:
                    mm2(n)
                n = g
                if 0 <= n < NTOT:
                    sim(n)
                for b in range(NB):
                    if g == 16 * b + 23:
                        tail(b)

        # ---------------- ACT ----------------
        @blk.scalar
        def _(s):
            def ex(n):
                b, i = divmod(n, NBLK)
                sl = n % 6
                if n >= 6:
                    s.wait_ge(pe_pt, n - 5)   # p_sb 6-deep WAR
                ac = s.activation(p_sb[n % 6][:], pS[:, sl, :], Exp,
                                  bias=NM[b % 2][:, i:i + 1])
                ac._wait_ge(dve_nm, 8 * b + i // 2 + 1)
                ac.then_inc(act_p, 1)

            def outcp_a(n):
                b, i = divmod(n, NBLK)
                ko = n % 3
                s.wait_ge(dve_rs, n + 1)
                if i == 0 and b >= 2:
                    s.wait_ge(s_out, 64 * (b - 1))
                s.mul(o_sb[b % 2][:, i, :], pO[ko][:, 0:256],
                      RS[b % 2][:, i:i + 1]).then_inc(act_oA, 1)

            def t1(b):
                if b == 0:
                    s.wait_ge(s_cin, _TH[(-1, "c100")])
                s.wait_ge(dve_nm, 8 * (b + 1))
                if b >= 2:
                    s.wait_ge(pt_, b - 1)     # tail(b-2) done reading E/esum
                s.activation(E_all[b % 2][:], NM[b % 2][:], Exp, bias=c100[:],
                             scale=-1.0, accum_out=esum[b % 2][:]).then_inc(at, 1)

            for g in range(NSLOT):
                n = g - L_OC
                if 0 <= n < NTOT and not outcp_on_dve(n):
                    outcp_a(n)
                n = g - L_EX
                if 0 <= n < NTOT:
                    ex(n)
                for b in range(NB):
                    if g == 16 * b + 21:
                        t1(b)

        # ---------------- DVE ----------------
        @blk.vector
        def _(v):
            def nm_pair(pg):
                b, pq = divmod(pg, 8)
                if pq == 0 and b >= 2:
                    v.wait_ge(at, b - 1)   # T1(b-2) done reading NM[b%2]
                base = (2 * pg) % 6
                rd = v.tensor_reduce(NM[b % 2][:, 2 * pq:2 * pq + 2],
                                     pS[:, base:base + 2, :], AX.X, OP.max,
                                     negate=True)
                rd._wait_ge(pe_s, 2 * pg + 2)
                rd.then_inc(dve_nm, 1)

            def ptr_pair(p):
                n1 = 2 * p + 1
                if p >= 2:
                    v.wait_ge(pe_o, n1 - 3)   # mm2s of pair evicted 2 pairs ago
                cp = v.tensor_copy(ptr[p % 2][:],
                                   pPT[p % 2][:].bitcast(BF16)[:, 0:512])
                cp._wait_ge(pe_pt, n1 + 1)
                cp.then_inc(dve_ptr, 1)

            def recip(n):
                b, i = divmod(n, NBLK)
                ko = n % 3
                if i == 0 and b >= 2:
                    v.wait_ge(act_oA, cnt_a(16 * (b - 1) - 1))   # RS[b%2] WAR
                    v.wait_ge(act_oD, cnt_d(16 * (b - 1) - 1))
                rc = v.reciprocal(RS[b % 2][:, i:i + 1], pO[ko][:, 256:257])
                rc._wait_ge(pe_o, n + 1)
                rc.then_inc(dve_rs, 1)

            def outcp_d(n):
                b, i = divmod(n, NBLK)
                ko = n % 3
                v.wait_ge(dve_rs, n + 1)
                v.tensor_scalar_mul(o_sb[b % 2][:, i, :], pO[ko][:, 0:256],
                                    RS[b % 2][:, i:i + 1]).then_inc(act_oD, 1)

            def qc_stage(b):
                v.wait_ge(pt_, b + 1)
                if b >= 2:
                    v.wait_ge(s_qc, 16 * (b - 1))    # qc DMA(b-2) done
                v.tensor_copy(qc_sb[b % 2][:, 0:2], pM[:, 300:302])
                v.tensor_copy(qc_sb[b % 2][0:1, 2:3],
                              pM[0:1, 310:311]).then_inc(dv_qc, 1)

            for g in range(NSLOT):
                if g >= 3 and (g - 3) % 2 == 0 and (g - 3) // 2 < NTOT // 2:
                    nm_pair((g - 3) // 2)
                if g >= 9 and g % 2 == 1 and (g - 9) // 2 < NTOT // 2:
                    ptr_pair((g - 9) // 2)
                n = g - L_RC
                if 0 <= n < NTOT:
                    recip(n)
                n = g - L_OC
                if 0 <= n < NTOT and outcp_on_dve(n):
                    outcp_d(n)
                for b in range(NB):
                    if g == 16 * b + 25:
                        qc_stage(b)

    return nc, es


_CACHE = {}


def _get_program():
    if "nc" not in _CACHE:
        nc, es = build_program()
        _CACHE["nc"] = nc
        _CACHE["es"] = es
    return _CACHE["nc"]


def kernel(context_repr, question_repr, context_len, question_len):
    C = np.ascontiguousarray(np.asarray(context_repr, np.float32))
    Q = np.ascontiguousarray(np.asarray(question_repr, np.float32))
    context_len = np.asarray(context_len, np.int32)
    question_len = np.asarray(question_len, np.int32)
    bf16 = ml_dtypes.bfloat16

    cm = (np.arange(TC)[None, :] < context_len[:, None]).astype(np.float32)
    qm = (np.arange(TQ)[None, :] < question_len[:, None]).astype(np.float32)
    mcf = np.stack([SQ * cm, np.ones_like(cm)], axis=1)
    mqf = np.stack([SQ * qm, np.full_like(qm, NEG)], axis=1)
    mskh = np.ascontiguousarray(np.concatenate([mqf, mcf], axis=2))

    ct = C.transpose(0, 2, 1).reshape(B, 2, 128, TC)
    qt = Q.transpose(0, 2, 1).reshape(B, 2, 128, TQ)
    ctq = np.ascontiguousarray(np.concatenate([qt, ct], axis=3))
    cn = C.astype(bf16)
    qn = np.concatenate([Q, np.ones((B, TQ, 1), np.float32)], axis=2)
    qn = np.ascontiguousarray(qn.reshape(B, 2, 128, QN_W).astype(bf16))
    identb = np.eye(128, dtype=bf16)
    c100 = np.full((128, 1), -100.0, np.float32)
    ones128 = np.ones((128, 1), np.float32)

    nc = _get_program()
    in_maps = []
    for core in range(NCORES):
        sl = slice(core * NB, (core + 1) * NB)
        in_maps.append({
            "ctq": np.ascontiguousarray(ctq[sl]),
            "cn": np.ascontiguousarray(cn[sl]),
            "qn": np.ascontiguousarray(qn[sl]),
            "msk": np.ascontiguousarray(mskh[sl]),
            "identb": identb,
            "c100": c100,
            "ones128": ones128,
        })

    res = run_bass_kernel_spmd(nc, in_maps, list(range(NCORES)))
    out1 = np.concatenate(
        [np.asarray(r["o"]).reshape(NB, TC, D).astype(np.float32)
         for r in res.results], axis=0)
    qc_raw = np.concatenate(
        [np.asarray(r["qc"]).reshape(NB, 128, 3) for r in res.results], axis=0)
    q2c = qc_raw[:, :, 0:2].transpose(0, 2, 1).reshape(B, D) / qc_raw[:, 0:1, 2]
    out2 = np.ascontiguousarray(np.broadcast_to(q2c[:, None, :], (B, TC, D)))
    return out1, out2
